# revision 38
# baseline (speedup 1.0000x reference)
"""Trainium2 Bass kernel for a 2-layer GCN (EnhancedGNN) with triple global
pooling and a final FC, run SPMD across 8 NeuronCores.

Strategy:
  - Nodes are re-ordered so every 128-row block belongs to exactly one graph
    ("pure blocks"), padded per-graph to multiples of 128. Blocks are sharded
    contiguously across the 8 cores (dst / data parallel).
  - Per layer: each core transforms its node shard (x @ W, scaled by
    dinv = deg^-1/2) into a bf16 "table" shard kept in SBUF AND pushed to
    DRAM; two AllGathers (one per half of the shard) replicate the table to
    every core so gathers on the first half can start while the second half
    is still in flight.
  - Aggregation: per group of 2 dst blocks and per table half, one
    dma_gather fetches the 512B message rows for all edges of the group
    (edges laid contiguously, slot-coded with a 256*block offset); messages
    are scattered into per-block PSUM accumulators with one-hot matmuls
    (B built on DVE via is_equal against offset iotas). Self-loop terms use
    an identity matmul against the SBUF-resident local table block (no
    gather, and it initializes the PSUM).
  - Pooling: per-graph sums via a one-hot matmul, per-graph max via
    per-block feature-major reduce_max + data-driven graph masks evaluated
    incrementally (as soon as a graph's last block is aggregated), then
    AllReduce(add/max) and a tiny FC run redundantly on every core.

The kernel program is identical on all 8 cores (SPMD); all per-core
differences live in the input data. Structure constants (tile counts etc.)
are maxima over cores so the program is uniform.
"""

import numpy as np
import ml_dtypes

import concourse.bass as bass
import concourse.tile as tile
from concourse import bacc, mybir
from concourse.bass_utils import run_bass_kernel_spmd

P = 128
NCORES = 8
GROUP_NBLK = 2  # dst blocks per gather group

BF16 = ml_dtypes.bfloat16
PAD_SLOT = 1000.0  # slot code that never matches any iota offset


def _cdiv(a, b):
    return -(-a // b)


# --------------------------------------------------------------------------
# Host-side preprocessing: sharding, edge grouping, auxiliary tensors.
# --------------------------------------------------------------------------

def preprocess(x, edge_index, batch, n_graphs, W1, b1, W2, b2, Wfc, bfc,
               n_cores=NCORES):
    x = np.asarray(x, np.float32)
    ei = np.asarray(edge_index, np.int64)
    batch = np.asarray(batch, np.int64)
    G = int(n_graphs)
    N = x.shape[0]
    F = x.shape[1]
    FH = W1.shape[1]
    FO = Wfc.shape[1]
    assert F == FH, "kernel assumes F_IN == F_HID"

    # degrees (dst side, + self loop), as in the reference
    deg = np.bincount(ei[1], minlength=N).astype(np.float32) + 1.0
    dinv = 1.0 / np.sqrt(deg)
    sqdeg = np.sqrt(deg)

    # --- graph-padded node ordering (pure blocks) ---
    cnt = np.bincount(batch, minlength=G).astype(np.int64)  # nodes per graph
    blocks_g = _cdiv(cnt, P)  # 0 for empty graphs
    total_blocks = int(blocks_g.sum())
    # pad so BPC is even (needed for the half-split of each core's shard)
    total_blocks_padded = _cdiv(total_blocks, 2 * n_cores) * 2 * n_cores
    BPC = total_blocks_padded // n_cores
    RPC = BPC * P
    NP = total_blocks_padded * P
    HROWS = RPC // 2            # local rows per table half
    HALF = NP // 2              # rows per table half (all cores)
    assert HALF <= 32768, f"table half {HALF} exceeds int16 index range"
    NGRP = BPC // GROUP_NBLK
    assert BPC % GROUP_NBLK == 0

    blk_start = np.concatenate([[0], np.cumsum(blocks_g)])  # per graph
    row_start = blk_start * P
    first_node = np.concatenate([[0], np.cumsum(cnt)])[:-1]
    new_pos = row_start[batch] + (np.arange(N) - first_node[batch])
    row2node = np.full(NP, -1, np.int64)
    row2node[new_pos] = np.arange(N)
    real = row2node >= 0

    # per padded row data
    x_pad = np.zeros((NP, F), np.float32)
    x_pad[real] = x[row2node[real]]
    dinv_pad = np.ones(NP, np.float32)
    dinv_pad[real] = dinv[row2node[real]]
    sqdeg_pad = np.zeros(NP, np.float32)
    sqdeg_pad[real] = sqdeg[row2node[real]]
    g_of_block = np.full(total_blocks_padded, -1, np.int64)
    for g in range(G):
        g_of_block[blk_start[g]:blk_start[g + 1]] = g

    # --- edges (real edges only; +I loops handled by identity matmuls) ---
    es = new_pos[ei[0]]
    ed = new_pos[ei[1]]
    core = ed // RPC
    pos = (ed % RPC) // P          # block position within core
    grp = pos // GROUP_NBLK
    pig = pos % GROUP_NBLK         # position in group
    slot = ed % P
    lr = es % RPC                  # src local row on its owner core
    half = lr // HROWS
    idx16 = (es // RPC) * HROWS + (lr - half * HROWS)

    # counts per (core, group, half)
    cnt3 = np.zeros((n_cores, NGRP, 2), np.int64)
    np.add.at(cnt3, (core, grp, half), 1)
    Tgh = _cdiv(cnt3.max(axis=0), P)  # [NGRP, 2] tiles, uniform across cores
    MAXT = max(1, int(Tgh.max()))

    # --- per-core edge index / slot arrays, call-ordered ---
    order = np.lexsort((slot, pig, half, grp, core))
    so = dict(core=core[order], grp=grp[order], half=half[order],
              pig=pig[order], slot=slot[order], idx16=idx16[order])
    run_start = np.zeros((n_cores, NGRP, 2), np.int64)
    flat_cnt = cnt3.reshape(-1)
    np.cumsum(flat_cnt[:-1], out=run_start.reshape(-1)[1:])

    # calls in (grp, half) order; record structure
    calls = []      # dicts: grp, h, T, tstart, idx_off
    tile_piggies = []  # per global tile: set of piggies present (union cores)
    tt = 0
    idxcols = 0
    for g in range(NGRP):
        for h in (0, 1):
            T = int(Tgh[g, h])
            if T == 0:
                continue
            calls.append(dict(grp=g, h=h, T=T, tstart=tt, idx_off=idxcols))
            for t in range(T):
                tile_piggies.append(set())
            tt += T
            idxcols += T * 8
    TT = tt
    IDXCOLS = idxcols
    call_of = {(c_["grp"], c_["h"]): i for i, c_ in enumerate(calls)}

    # trailing -1 pads are skipped by the gather ucode; per-core valid
    # counts are read at runtime (value_load) into num_idxs_reg
    idxflat = np.full((n_cores, TT * P), -1, np.int16)
    counts = np.ones((n_cores, max(1, len(calls))), np.int32)
    for c in range(n_cores):
        for ci, call in enumerate(calls):
            g, h, T = call["grp"], call["h"], call["T"]
            n = int(cnt3[c, g, h])
            s0 = int(run_start[c, g, h])
            o = call["tstart"] * P
            idxflat[c, o:o + n] = so["idx16"][s0:s0 + n].astype(np.int16)
            if n == 0:  # keep >=1 valid index (ucode/sim requirement)
                idxflat[c, o] = 0
            counts[c, ci] = max(n, 1)
            for t in range(T):
                a, b_ = t * P, min((t + 1) * P, n)
                if a >= n:
                    break
                pres = np.unique(so["pig"][s0 + a:s0 + b_])
                tile_piggies[call["tstart"] + t].update(int(p) for p in pres)

    for t in range(TT):
        if not tile_piggies[t]:
            tile_piggies[t].add(0)

    # matmul ids: one host-built one-hot panel per (tile, pig), grouped by grp
    mmid = np.full((TT, GROUP_NBLK), -1, np.int64)
    grp_mms = [[] for _ in range(NGRP)]  # (ci, t, gt, pig, m)
    m = 0
    for ci, call in enumerate(calls):
        for t in range(call["T"]):
            gt = call["tstart"] + t
            for pig in sorted(tile_piggies[gt]):
                mmid[gt, pig] = m
                grp_mms[call["grp"]].append((ci, t, gt, pig, m))
                m += 1
    NMM = m
    grp_moff = [min((e[4] for e in g_), default=0) for g_ in grp_mms]
    MAXM = max(len(g_) for g_ in grp_mms)

    # wrap-16 + replicate-to-128 index layout, call-local
    gidx = np.zeros((n_cores, P, IDXCOLS), np.int16)
    for call in calls:
        a = call["tstart"] * P
        T = call["T"]
        region = idxflat[:, a:a + T * P]
        arr = region.reshape(n_cores, T * 8, 16).transpose(0, 2, 1)
        gidx[:, :, call["idx_off"]:call["idx_off"] + T * 8] = (
            np.tile(arr, (1, 8, 1)))

    # host-built one-hot scatter panels (shared by both layers)
    bpans = []
    for c in range(n_cores):
        bp = np.zeros((P, NMM * P), BF16)
        for ci, call in enumerate(calls):
            g, h = call["grp"], call["h"]
            n = int(cnt3[c, g, h])
            if n == 0:
                continue
            s0 = int(run_start[c, g, h])
            p_ = np.arange(n)
            gt = call["tstart"] + p_ // P
            sp = p_ % P
            pig_e = so["pig"][s0:s0 + n]
            slot_e = so["slot"][s0:s0 + n]
            m_e = mmid[gt, pig_e]
            bp[sp, m_e * P + slot_e] = 1.0
        bpans.append(bp)

    # --- pooling helpers ---
    rows = np.arange(NP)
    rcore = rows // RPC
    rblk = (rows % RPC) // P
    rslot = rows % P
    pm = np.zeros((n_cores, P, BPC * G), BF16)
    rg = np.where(real, batch[np.clip(row2node, 0, N - 1)], -1)
    val = real
    pm[rcore[val], rslot[val], rblk[val] * G + rg[val]] = 1.0
    pmask = np.zeros((n_cores, P, G * BPC), BF16)
    for c in range(n_cores):
        for b in range(BPC):
            g = g_of_block[c * BPC + b]
            if g >= 0:
                pmask[c, :, g * BPC + b] = 1.0
    recip = (1.0 / np.maximum(cnt, 1.0)).astype(np.float32).reshape(G, 1)
    # last block position (over all cores) of each graph; empty graphs -> 0
    lastpos = np.zeros(G, np.int64)
    for bid in range(total_blocks_padded):
        g = g_of_block[bid]
        if g >= 0:
            lastpos[g] = max(lastpos[g], bid % BPC)
    graphs_at = [[] for _ in range(BPC)]
    for g in range(G):
        graphs_at[int(lastpos[g])].append(g)

    # --- per-core input maps ---
    in_maps = []
    for c in range(n_cores):
        r0, r1 = c * RPC, (c + 1) * RPC
        m = {
            "xt": np.ascontiguousarray(x_pad[r0:r1].T).astype(BF16),
            "w1": np.asarray(W1, np.float32).astype(BF16),
            "w2": np.asarray(W2, np.float32).astype(BF16),
            "wfc": np.asarray(Wfc, np.float32).astype(BF16),
            "b1r": np.asarray(b1, np.float32).reshape(1, FH).astype(BF16),
            "b2r": np.asarray(b2, np.float32).reshape(1, FH).astype(BF16),
            "bfcr": np.asarray(bfc, np.float32).reshape(1, FO).astype(BF16),
            "sqdeg": sqdeg_pad[r0:r1].reshape(1, RPC).astype(BF16),
            "dinv": np.ascontiguousarray(
                dinv_pad[r0:r1].reshape(BPC, P).T).astype(np.float32),
            "gidx": gidx[c],
            "counts": counts[c:c + 1],
            "bpan": bpans[c],
            "pm": pm[c],
            "pmask": pmask[c],
            "recip": recip,
        }
        in_maps.append(m)

    plan = dict(
        G=G, F=F, FH=FH, FO=FO, BPC=BPC, RPC=RPC, NP=NP, HALF=HALF,
        HROWS=HROWS, NGRP=NGRP, TT=TT, IDXCOLS=IDXCOLS, MAXT=MAXT,
        calls=calls, call_of=call_of,
        grp_mms=grp_mms, grp_moff=grp_moff, NMM=NMM, MAXM=MAXM,
        graphs_at=graphs_at,
        n_cores=n_cores,
        has_b1=bool(np.any(np.asarray(b1))),
        has_b2=bool(np.any(np.asarray(b2))),
        has_bfc=bool(np.any(np.asarray(bfc))),
    )
    return plan, in_maps


# --------------------------------------------------------------------------
# Bass program builder (identical on all cores).
# --------------------------------------------------------------------------

def build(plan, debug=False):
    dt = mybir.dt
    G, F, FH, FO = plan["G"], plan["F"], plan["FH"], plan["FO"]
    BPC, RPC, NP = plan["BPC"], plan["RPC"], plan["NP"]
    HROWS, NGRP = plan["HROWS"], plan["NGRP"]
    TT, IDXCOLS, MAXT = plan["TT"], plan["IDXCOLS"], plan["MAXT"]
    calls, call_of = plan["calls"], plan["call_of"]
    grp_mms, grp_moff = plan["grp_mms"], plan["grp_moff"]
    NMM, MAXM = plan["NMM"], plan["MAXM"]
    graphs_at = plan["graphs_at"]
    n_cores = plan["n_cores"]
    KC = F // P          # k-chunks for the transforms (2)
    FCK = (3 * FH) // P  # k-chunks for the FC (6)
    HB = BPC // 2        # blocks per table half

    nc = bacc.Bacc("TRN2", target_bir_lowering=False, debug=debug,
                   num_devices=n_cores)

    def din(name, shape, dtype):
        return nc.dram_tensor(name, shape, dtype, kind="ExternalInput").ap()

    xt_d = din("xt", [F, RPC], dt.bfloat16)
    w1_d = din("w1", [F, FH], dt.bfloat16)
    w2_d = din("w2", [FH, FH], dt.bfloat16)
    wfc_d = din("wfc", [3 * FH, FO], dt.bfloat16)
    b1r_d = din("b1r", [1, FH], dt.bfloat16)
    b2r_d = din("b2r", [1, FH], dt.bfloat16)
    bfcr_d = din("bfcr", [1, FO], dt.bfloat16)
    sqdeg_d = din("sqdeg", [1, RPC], dt.bfloat16)
    dinv_d = din("dinv", [P, BPC], dt.float32)
    gidx_d = din("gidx", [P, IDXCOLS], dt.int16)
    counts_d = din("counts", [1, max(1, len(calls))], dt.int32)
    bpan_d = din("bpan", [P, NMM * P], dt.bfloat16)
    pm_d = din("pm", [P, BPC * G], dt.bfloat16)
    pmask_d = din("pmask", [P, G * BPC], dt.bfloat16)
    recip_d = din("recip", [G, 1], dt.float32)
    out_d = nc.dram_tensor("out", [G, FO], dt.float32,
                           kind="ExternalOutput").ap()

    rg = [list(range(n_cores))]

    from contextlib import ExitStack
    with tile.TileContext(nc) as tc, ExitStack() as ctx:
        const = ctx.enter_context(tc.tile_pool(name="const", bufs=1))
        dram = ctx.enter_context(tc.tile_pool(name="dram", bufs=1, space="DRAM"))
        tfpsum = ctx.enter_context(tc.tile_pool(name="tfpsum", bufs=2, space="PSUM"))
        aggpsum = ctx.enter_context(tc.tile_pool(name="aggpsum", bufs=3, space="PSUM"))
        tpsum = ctx.enter_context(tc.tile_pool(name="tpsum", bufs=1, space="PSUM"))
        spsum = ctx.enter_context(tc.tile_pool(name="spsum", bufs=1, space="PSUM"))
        fcpsum = ctx.enter_context(tc.tile_pool(name="fcpsum", bufs=1, space="PSUM"))
        msgp = ctx.enter_context(tc.tile_pool(name="msgp", bufs=6))
        xtp = ctx.enter_context(tc.tile_pool(name="xtp", bufs=2))
        bpp = ctx.enter_context(tc.tile_pool(name="bpp", bufs=2))
        btp = ctx.enter_context(tc.tile_pool(name="btp", bufs=4))
        hp = ctx.enter_context(tc.tile_pool(name="hp", bufs=3))
        htp = ctx.enter_context(tc.tile_pool(name="htp", bufs=4))
        tailp = ctx.enter_context(tc.tile_pool(name="tailp", bufs=1))

        # ---------------- constants into SBUF ----------------
        def cload(tag, dram_ap, shape, dtype):
            t = const.tile(shape, dtype, tag=tag)
            nc.sync.dma_start(out=t[:], in_=dram_ap)
            return t

        # trigger the first (dummy) collective ASAP: the runtime's one-time
        # collective init (~60us) runs serially before the first real
        # AllGather otherwise
        warm_in = dram.tile([8, 16], dt.bfloat16, tag="warmin")
        warm_out = dram.tile([64, 16], dt.bfloat16, tag="warmout",
                             addr_space="Shared")
        wz = const.tile([8, 16], dt.bfloat16, tag="wz")
        nc.gpsimd.memset(wz[:], 0.0)
        nc.sync.dma_start(out=warm_in[:], in_=wz[:])
        nc.gpsimd.collective_compute(
            "AllGather", mybir.AluOpType.bypass,
            ins=[warm_in[:].opt()], outs=[warm_out[:].opt()],
            replica_groups=rg)

        NXT = next(n for n in (6, 4, 3, 2, 1) if BPC % n == 0)
        XBL = BPC // NXT
        w_sb = []
        for tag, d in (("w1", w1_d), ("w2", w2_d)):
            t = const.tile([P, KC * FH], dt.bfloat16, tag=tag)
            for c in range(KC):
                nc.sync.dma_start(out=t[:, c * FH:(c + 1) * FH],
                                  in_=d[c * P:(c + 1) * P, :])
            w_sb.append(t)
        wfc_sb = const.tile([P, FCK * FO], dt.bfloat16, tag="wfc")
        for c in range(FCK):
            nc.sync.dma_start(out=wfc_sb[:, c * FO:(c + 1) * FO],
                              in_=wfc_d[c * P:(c + 1) * P, :])
        dinv_sb = cload("dinv", dinv_d, [P, BPC], dt.float32)

        iota_sb = const.tile([P, P], dt.float32, tag="iota")
        nc.gpsimd.iota(out=iota_sb[:], pattern=[[1, P]], base=0,
                       channel_multiplier=0,
                       allow_small_or_imprecise_dtypes=True)
        iotac_sb = const.tile([P, 1], dt.float32, tag="iotac")
        nc.gpsimd.iota(out=iotac_sb[:], pattern=[[0, 1]], base=0,
                       channel_multiplier=1,
                       allow_small_or_imprecise_dtypes=True)
        ident_sb = const.tile([P, P], dt.bfloat16, tag="ident")
        nc.vector.tensor_tensor(out=ident_sb[:],
                                in0=iotac_sb[:].to_broadcast([P, P]),
                                in1=iota_sb[:],
                                op=mybir.AluOpType.is_equal)
        ones_sb = const.tile([1, G], dt.bfloat16, tag="ones")
        nc.gpsimd.memset(ones_sb[:], 1.0)
        blockmax = const.tile([P, KC * BPC], dt.bfloat16, tag="bmax")
        nc.gpsimd.memset(blockmax[:], 0.0)
        # persistent per-layer local tables (bf16, dinv-scaled)
        tbl = [const.tile([P, BPC * FH], dt.bfloat16, name=f"tbl{l}",
                          tag=f"tbl{l}")
               for l in range(2)]

        # DRAM bounce buffers for collectives (per layer, per half)
        ag_in = [[dram.tile([HROWS, FH], dt.bfloat16, name=f"agin{l}{h}",
                            tag=f"agin{l}{h}")
                  for h in (0, 1)] for l in range(2)]
        ag_out = [[dram.tile([HROWS * n_cores, FH], dt.bfloat16,
                             name=f"agout{l}{h}", tag=f"agout{l}{h}")
                   for h in (0, 1)] for l in range(2)]
        ars_in = [dram.tile([G, FH], dt.bfloat16, name=f"arsin{k}",
                            tag=f"arsin{k}") for k in (0, 1)]
        ars_out = [dram.tile([G, FH], dt.bfloat16, name=f"arsout{k}",
                             tag=f"arsout{k}", addr_space="Shared")
                   for k in (0, 1)]
        arm_in = dram.tile([P, KC * G], dt.bfloat16, tag="armin")
        arm_out = dram.tile([P, KC * G], dt.bfloat16, tag="armout",
                            addr_space="Shared")

        Copy = mybir.ActivationFunctionType.Copy
        Relu = mybir.ActivationFunctionType.Relu

        cnt_reg = nc.gpsimd.alloc_register("cntreg")

        # zero-fill the msg ring once: slots of skipped (-1 pad) gather
        # indices are never written, and must read as finite for the
        # zero panel rows to nullify them
        for _ in range(6):
            mb0 = msgp.tile([P, MAXT * FH], dt.bfloat16, tag="msg")
            nc.gpsimd.memset(mb0[:], 0.0)

        def allgather(l, h):
            nc.gpsimd.collective_compute(
                "AllGather", mybir.AluOpType.bypass,
                ins=[ag_in[l][h][:].opt()], outs=[ag_out[l][h][:].opt()],
                replica_groups=rg)

        def push_block(l, b):
            h = 0 if b < HB else 1
            r0 = (b - h * HB) * P
            nc.scalar.dma_start(out=ag_in[l][h][r0:r0 + P, :],
                                in_=tbl[l][:, b * FH:(b + 1) * FH])

        # ---------------- layer-1 transform ----------------
        xt_ch = None
        for b in range(BPC):
            if b % XBL == 0:
                xt_ch = xtp.tile([P, KC * XBL * P], dt.bfloat16, tag="xtch")
                for c in range(KC):
                    nc.sync.dma_start(
                        out=xt_ch[:, c * XBL * P:(c + 1) * XBL * P],
                        in_=xt_d[c * P:(c + 1) * P,
                                 (b // XBL) * XBL * P:
                                 (b // XBL + 1) * XBL * P])
            bo = b % XBL
            ps = tfpsum.tile([P, FH], dt.float32, tag="tfps")
            for c in range(KC):
                nc.tensor.matmul(
                    out=ps[:],
                    lhsT=xt_ch[:, c * XBL * P + bo * P:
                               c * XBL * P + (bo + 1) * P],
                    rhs=w_sb[0][:, c * FH:(c + 1) * FH],
                    start=(c == 0), stop=(c == KC - 1))
            nc.scalar.activation(out=tbl[0][:, b * FH:(b + 1) * FH],
                                 in_=ps[:], func=Copy,
                                 scale=dinv_sb[:, b:b + 1])
            push_block(0, b)
            if b == HB - 1:
                allgather(0, 0)
        allgather(0, 1)

        # deferred const loads (needed from the agg phase on, not by the
        # transforms -- keeps the early sync-DMA queue clear so the first
        # AllGather can fire as soon as the transforms finish)
        gidx_sb = cload("gidx", gidx_d, [P, IDXCOLS], dt.int16)
        counts_sb = cload("counts", counts_d,
                          [1, max(1, len(calls))], dt.int32)
        b1r_sb = cload("b1r", b1r_d, [1, FH], dt.bfloat16)
        b2r_sb = cload("b2r", b2r_d, [1, FH], dt.bfloat16)
        bfcr_sb = cload("bfcr", bfcr_d, [1, FO], dt.bfloat16)
        sqdeg_sb = cload("sqdeg", sqdeg_d, [1, RPC], dt.bfloat16)
        pm_sb = cload("pm", pm_d, [P, BPC * G], dt.bfloat16)
        pmask_sb = cload("pmask", pmask_d, [P, G * BPC], dt.bfloat16)
        recip_sb = cload("recip", recip_d, [G, 1], dt.float32)

        # ---------------- aggregation over edges ----------------
        AWIN = 3  # h=0 calls issued ahead of h=1 calls (hides AG of half 1)

        def agg_layer(l, bias_row, has_bias, produce_block):
            # interleave calls: [g0A g1A g2A g0B g1B ...] with groups'
            # matmul work following once both halves of a group are in.
            mbufs = {}   # call index -> sbuf tile

            def do_call(ci):
                call = calls[ci]
                T = call["T"]
                nc.gpsimd.reg_load(cnt_reg, counts_sb[0:1, ci:ci + 1])
                mb = msgp.tile([P, MAXT * FH], dt.bfloat16, tag="msg")
                out_ap = mb[:, :T * FH].rearrange("p (t e) -> p t e", e=FH)
                nc.gpsimd.dma_gather(
                    out_ap=out_ap,
                    in_ap=ag_out[l][call["h"]][:],
                    idxs_ap=gidx_sb[:, call["idx_off"]:
                                    call["idx_off"] + T * 8],
                    num_idxs=T * P,
                    num_idxs_reg=cnt_reg,
                    elem_size=FH,
                    single_packet=False)
                mbufs[ci] = mb

            order = []   # call issue order (A-window interleave)
            apend = [ci for ci in range(len(calls)) if calls[ci]["h"] == 0]
            bpend = [ci for ci in range(len(calls)) if calls[ci]["h"] == 1]
            ai = bi = 0
            while ai < len(apend) or bi < len(bpend):
                win = AWIN + 1 if bi == 0 else AWIN
                if ai < len(apend) and (ai - bi < win or bi >= len(bpend)):
                    order.append(apend[ai]); ai += 1
                else:
                    order.append(bpend[bi]); bi += 1

            done_upto = 0  # groups fully processed

            def group_ready(g):
                for h in (0, 1):
                    ci = call_of.get((g, h))
                    if ci is not None and ci not in mbufs:
                        return False
                return True

            def process_group(g):
                nmm = len(grp_mms[g])
                moff = grp_moff[g]
                bsl = None
                if nmm:
                    bsl = bpp.tile([P, MAXM * P], dt.bfloat16, tag="bsl")
                    nc.sync.dma_start(
                        out=bsl[:, :nmm * P],
                        in_=bpan_d[:, moff * P:(moff + nmm) * P])
                for pig in range(GROUP_NBLK):
                    b = g * GROUP_NBLK + pig
                    ps = aggpsum.tile([P, FH], dt.float32, tag="aggps")
                    mms = [e for e in grp_mms[g] if e[3] == pig]
                    # self loop: identity matmul against local table block
                    nc.tensor.matmul(out=ps[:], lhsT=ident_sb[:],
                                     rhs=tbl[l][:, b * FH:(b + 1) * FH],
                                     start=True,
                                     stop=(not mms) and not has_bias)
                    for k, (ci, t, gt, _pig, m) in enumerate(mms):
                        mo = m - moff
                        nc.tensor.matmul(
                            out=ps[:], lhsT=bsl[:, mo * P:(mo + 1) * P],
                            rhs=mbufs[ci][:, t * FH:(t + 1) * FH],
                            start=False,
                            stop=(k == len(mms) - 1) and not has_bias)
                    if has_bias:
                        nc.tensor.matmul(
                            out=ps[:],
                            lhsT=sqdeg_sb[:, b * P:(b + 1) * P],
                            rhs=bias_row[:],
                            start=False, stop=True)
                    produce_block(b, ps)

            for ci in order:
                do_call(ci)
                # process any groups that are now complete, in order
                while done_upto < NGRP and group_ready(done_upto):
                    process_group(done_upto)
                    g = done_upto
                    done_upto += 1
                    # release msg buffers of this group
                    for h in (0, 1):
                        cix = call_of.get((g, h))
                        if cix in mbufs:
                            del mbufs[cix]
                    if l == 0 and g == min(NGRP - 1, HB // GROUP_NBLK + 2):
                        allgather(1, 0)
            while done_upto < NGRP:
                process_group(done_upto)
                done_upto += 1
            if l == 0:
                allgather(1, 1)

        # layer-1 block epilogue: relu, transform to layer-2 table
        def produce1(b, ps):
            h1 = hp.tile([P, FH], dt.bfloat16, tag="h1")
            nc.scalar.activation(out=h1[:], in_=ps[:], func=Relu,
                                 scale=dinv_sb[:, b:b + 1])
            h1t = []
            for c in range(KC):
                tp = tpsum.tile([P, P], dt.bfloat16, tag="tp")
                nc.tensor.transpose(out=tp[:],
                                    in_=h1[:, c * P:(c + 1) * P],
                                    identity=ident_sb[:])
                ht = htp.tile([P, P], dt.bfloat16, tag="ht")
                nc.vector.tensor_copy(out=ht[:], in_=tp[:])
                h1t.append(ht)
            ps2 = tfpsum.tile([P, FH], dt.float32, tag="tfps")
            for c in range(KC):
                nc.tensor.matmul(out=ps2[:], lhsT=h1t[c][:],
                                 rhs=w_sb[1][:, c * FH:(c + 1) * FH],
                                 start=(c == 0), stop=(c == KC - 1))
            nc.scalar.activation(out=tbl[1][:, b * FH:(b + 1) * FH],
                                 in_=ps2[:], func=Copy,
                                 scale=dinv_sb[:, b:b + 1])
            push_block(1, b)

        agg_layer(0, b1r_sb, plan["has_b1"], produce1)

        # layer-2 block epilogue: relu, pooling contributions
        sums_ps = spsum.tile([G, FH], dt.float32, tag="sums")
        mxT_loc = const.tile([P, KC * G], dt.bfloat16, tag="mxT_loc")

        SUMS_SPLIT = BPC - 6 if BPC > 8 else -1  # no split on tiny configs

        def produce2(b, ps):
            h2 = hp.tile([P, FH], dt.bfloat16, tag="h2")
            nc.scalar.activation(out=h2[:], in_=ps[:], func=Relu,
                                 scale=dinv_sb[:, b:b + 1])
            nc.tensor.matmul(out=sums_ps[:],
                             lhsT=pm_sb[:, b * G:(b + 1) * G],
                             rhs=h2[:],
                             start=(b == 0 or b == SUMS_SPLIT),
                             stop=(b == SUMS_SPLIT - 1 or b == BPC - 1))
            if b == SUMS_SPLIT - 1:
                ssA = tailp.tile([G, FH], dt.bfloat16, tag="ssA")
                nc.vector.tensor_copy(out=ssA[:], in_=sums_ps[:])
                nc.sync.dma_start(out=ars_in[0][:], in_=ssA[:])
                nc.gpsimd.collective_compute(
                    "AllReduce", mybir.AluOpType.add,
                    ins=[ars_in[0][:].opt()], outs=[ars_out[0][:].opt()],
                    replica_groups=rg)
            for c in range(KC):
                tp = tpsum.tile([P, P], dt.bfloat16, tag="tp")
                nc.tensor.transpose(out=tp[:],
                                    in_=h2[:, c * P:(c + 1) * P],
                                    identity=ident_sb[:])
                nc.vector.tensor_reduce(
                    out=blockmax[:, c * BPC + b:c * BPC + b + 1],
                    in_=tp[:], axis=mybir.AxisListType.X,
                    op=mybir.AluOpType.max)
            # per-graph local max for graphs whose last block is b
            for g in graphs_at[b]:
                mtmp = btp.tile([P, BPC], dt.bfloat16, tag="mtmp")
                for c in range(KC):
                    nc.vector.tensor_tensor(
                        out=mtmp[:],
                        in0=blockmax[:, c * BPC:(c + 1) * BPC],
                        in1=pmask_sb[:, g * BPC:(g + 1) * BPC],
                        op=mybir.AluOpType.mult)
                    nc.vector.tensor_reduce(
                        out=mxT_loc[:, c * G + g:c * G + g + 1],
                        in_=mtmp[:],
                        axis=mybir.AxisListType.X, op=mybir.AluOpType.max)

        agg_layer(1, b2r_sb, plan["has_b2"], produce2)

        # ---------------- pooling tail ----------------
        sums_sb = tailp.tile([G, FH], dt.bfloat16, tag="sums_sb")
        nc.vector.tensor_copy(out=sums_sb[:], in_=sums_ps[:])
        nc.sync.dma_start(out=ars_in[1][:], in_=sums_sb[:])
        nc.gpsimd.collective_compute(
            "AllReduce", mybir.AluOpType.add,
            ins=[ars_in[1][:].opt()], outs=[ars_out[1][:].opt()],
            replica_groups=rg)
        nc.sync.dma_start(out=arm_in[:], in_=mxT_loc[:])
        nc.gpsimd.collective_compute(
            "AllReduce", mybir.AluOpType.max,
            ins=[arm_in[:].opt()], outs=[arm_out[:].opt()],
            replica_groups=rg)

        gsA = tailp.tile([G, FH], dt.bfloat16, tag="gsA")
        if SUMS_SPLIT > 0:
            nc.sync.dma_start(out=gsA[:], in_=ars_out[0][:])
        else:
            nc.gpsimd.memset(gsA[:], 0.0)
        gsB = tailp.tile([G, FH], dt.bfloat16, tag="gsB")
        nc.sync.dma_start(out=gsB[:], in_=ars_out[1][:])
        gsums = tailp.tile([G, FH], dt.bfloat16, tag="gsums")
        nc.vector.tensor_tensor(out=gsums[:], in0=gsA[:], in1=gsB[:],
                                op=mybir.AluOpType.add)
        mxT = tailp.tile([P, KC * G], dt.bfloat16, tag="mxT")
        nc.sync.dma_start(out=mxT[:], in_=arm_out[:])

        # mean / sums in bf16, transposed to feature-major for the FC
        mean_sb = tailp.tile([G, FH], dt.bfloat16, tag="mean")
        nc.vector.tensor_scalar(out=mean_sb[:], in0=gsums[:],
                                scalar1=recip_sb[:], scalar2=None,
                                op0=mybir.AluOpType.mult)
        sums_bf = tailp.tile([G, FH], dt.bfloat16, tag="sumsbf")
        nc.vector.tensor_copy(out=sums_bf[:], in_=gsums[:])
        meanT = tailp.tile([P, KC * G], dt.bfloat16, tag="meanT")
        sumsT = tailp.tile([P, KC * G], dt.bfloat16, tag="sumsT")
        for src, dst_t in ((mean_sb, meanT), (sums_bf, sumsT)):
            for c in range(KC):
                tp = tpsum.tile([P, P], dt.bfloat16, tag="tp")
                nc.tensor.transpose(out=tp[:, :G],
                                    in_=src[:, c * P:(c + 1) * P],
                                    identity=ident_sb[:G, :G])
                nc.vector.tensor_copy(out=dst_t[:, c * G:(c + 1) * G],
                                      in_=tp[:, :G])

        # final FC: out = [mean | max | sums] @ Wfc + bfc
        fc_ps = fcpsum.tile([G, FO], dt.float32, tag="fc")
        gT = [meanT, mxT, sumsT]
        k = 0
        for part in range(3):
            for c in range(KC):
                nc.tensor.matmul(
                    out=fc_ps[:], lhsT=gT[part][:, c * G:(c + 1) * G],
                    rhs=wfc_sb[:, k * FO:(k + 1) * FO],
                    start=(k == 0),
                    stop=(k == FCK - 1) and not plan["has_bfc"])
                k += 1
        if plan["has_bfc"]:
            nc.tensor.matmul(out=fc_ps[:], lhsT=ones_sb[:], rhs=bfcr_sb[:],
                             start=False, stop=True)
        out_sb = tailp.tile([G, FO], dt.float32, tag="out_sb")
        nc.vector.tensor_copy(out=out_sb[:], in_=fc_ps[:])
        nc.sync.dma_start(out=out_d[:], in_=out_sb[:])

    nc.compile()
    return nc


# --------------------------------------------------------------------------
# Entry point for the grading harness.
# --------------------------------------------------------------------------

def kernel(x, edge_index, batch, n_graphs, W1, b1, W2, b2, Wfc, bfc,
           **_unused):
    plan, in_maps = preprocess(x, edge_index, batch, n_graphs,
                               W1, b1, W2, b2, Wfc, bfc)
    nc = build(plan)
    res = run_bass_kernel_spmd(nc, in_maps, core_ids=list(range(NCORES)))
    out = np.asarray(res.results[0]["out"], np.float32)
    return out


# revision 40
# speedup vs baseline: 1.1222x; 1.1222x over previous
"""Trainium2 Bass kernel for a 2-layer GCN (EnhancedGNN) with triple global
pooling and a final FC, run SPMD across 8 NeuronCores.

Strategy:
  - Nodes are re-ordered so every 128-row block belongs to exactly one graph
    ("pure blocks"), padded per-graph to multiples of 128. Blocks are sharded
    contiguously across the 8 cores (dst / data parallel).
  - Per layer: each core transforms its node shard (x @ W, scaled by
    dinv = deg^-1/2) into a bf16 "table" shard kept in SBUF AND pushed to
    DRAM; two AllGathers (one per half of the shard) replicate the table to
    every core so gathers on the first half can start while the second half
    is still in flight.
  - Aggregation: per group of 2 dst blocks and per table half, one
    dma_gather fetches the 512B message rows for all edges of the group
    (edges laid contiguously, slot-coded with a 256*block offset); messages
    are scattered into per-block PSUM accumulators with one-hot matmuls
    (B built on DVE via is_equal against offset iotas). Self-loop terms use
    an identity matmul against the SBUF-resident local table block (no
    gather, and it initializes the PSUM).
  - Pooling: per-graph sums via a one-hot matmul, per-graph max via
    per-block feature-major reduce_max + data-driven graph masks evaluated
    incrementally (as soon as a graph's last block is aggregated), then
    AllReduce(add/max) and a tiny FC run redundantly on every core.

The kernel program is identical on all 8 cores (SPMD); all per-core
differences live in the input data. Structure constants (tile counts etc.)
are maxima over cores so the program is uniform.
"""

import numpy as np
import ml_dtypes

import concourse.bass as bass
import concourse.tile as tile
from concourse import bacc, mybir
from concourse.bass_utils import run_bass_kernel_spmd

P = 128
NCORES = 8
GROUP_NBLK = 2  # dst blocks per gather group

BF16 = ml_dtypes.bfloat16
PAD_SLOT = 1000.0  # slot code that never matches any iota offset


def _cdiv(a, b):
    return -(-a // b)


# --------------------------------------------------------------------------
# Host-side preprocessing: sharding, edge grouping, auxiliary tensors.
# --------------------------------------------------------------------------

def preprocess(x, edge_index, batch, n_graphs, W1, b1, W2, b2, Wfc, bfc,
               n_cores=NCORES):
    x = np.asarray(x, np.float32)
    ei = np.asarray(edge_index, np.int64)
    batch = np.asarray(batch, np.int64)
    G = int(n_graphs)
    N = x.shape[0]
    F = x.shape[1]
    FH = W1.shape[1]
    FO = Wfc.shape[1]
    assert F == FH, "kernel assumes F_IN == F_HID"

    # degrees (dst side, + self loop), as in the reference
    deg = np.bincount(ei[1], minlength=N).astype(np.float32) + 1.0
    dinv = 1.0 / np.sqrt(deg)
    sqdeg = np.sqrt(deg)

    # --- graph-padded node ordering (pure blocks) ---
    cnt = np.bincount(batch, minlength=G).astype(np.int64)  # nodes per graph
    blocks_g = _cdiv(cnt, P)  # 0 for empty graphs
    total_blocks = int(blocks_g.sum())
    # pad so BPC is even (needed for the half-split of each core's shard)
    total_blocks_padded = _cdiv(total_blocks, 2 * n_cores) * 2 * n_cores
    BPC = total_blocks_padded // n_cores
    RPC = BPC * P
    NP = total_blocks_padded * P
    HROWS = RPC // 2            # local rows per table half
    HALF = NP // 2              # rows per table half (all cores)
    assert HALF <= 32768, f"table half {HALF} exceeds int16 index range"
    NGRP = BPC // GROUP_NBLK
    assert BPC % GROUP_NBLK == 0

    blk_start = np.concatenate([[0], np.cumsum(blocks_g)])  # per graph
    row_start = blk_start * P
    first_node = np.concatenate([[0], np.cumsum(cnt)])[:-1]
    new_pos = row_start[batch] + (np.arange(N) - first_node[batch])
    row2node = np.full(NP, -1, np.int64)
    row2node[new_pos] = np.arange(N)
    real = row2node >= 0

    # per padded row data
    x_pad = np.zeros((NP, F), np.float32)
    x_pad[real] = x[row2node[real]]
    dinv_pad = np.ones(NP, np.float32)
    dinv_pad[real] = dinv[row2node[real]]
    sqdeg_pad = np.zeros(NP, np.float32)
    sqdeg_pad[real] = sqdeg[row2node[real]]
    g_of_block = np.full(total_blocks_padded, -1, np.int64)
    for g in range(G):
        g_of_block[blk_start[g]:blk_start[g + 1]] = g

    # --- edges (real edges only; +I loops handled by identity matmuls) ---
    es = new_pos[ei[0]]
    ed = new_pos[ei[1]]
    core = ed // RPC
    pos = (ed % RPC) // P          # block position within core
    grp = pos // GROUP_NBLK
    pig = pos % GROUP_NBLK         # position in group
    slot = ed % P
    lr = es % RPC                  # src local row on its owner core
    half = lr // HROWS
    idx16 = (es // RPC) * HROWS + (lr - half * HROWS)

    # counts per (core, group, half)
    cnt3 = np.zeros((n_cores, NGRP, 2), np.int64)
    np.add.at(cnt3, (core, grp, half), 1)
    Tgh = _cdiv(cnt3.max(axis=0), P)  # [NGRP, 2] tiles, uniform across cores
    MAXT = max(1, int(Tgh.max()))

    # --- per-core edge index / slot arrays, call-ordered ---
    # within each (core, group, half, pig) run, order edges by ascending
    # table index: the gather's 512B HBM reads become monotonic and
    # duplicate rows adjacent (the one-hot panels encode slots per
    # position, so slot order inside a run is free)
    order = np.lexsort((idx16, pig, half, grp, core))
    so = dict(core=core[order], grp=grp[order], half=half[order],
              pig=pig[order], slot=slot[order], idx16=idx16[order])
    run_start = np.zeros((n_cores, NGRP, 2), np.int64)
    flat_cnt = cnt3.reshape(-1)
    np.cumsum(flat_cnt[:-1], out=run_start.reshape(-1)[1:])

    # calls in (grp, half) order; record structure
    calls = []      # dicts: grp, h, T, tstart, idx_off
    tile_piggies = []  # per global tile: set of piggies present (union cores)
    tt = 0
    idxcols = 0
    for g in range(NGRP):
        for h in (0, 1):
            T = int(Tgh[g, h])
            if T == 0:
                continue
            calls.append(dict(grp=g, h=h, T=T, tstart=tt, idx_off=idxcols))
            for t in range(T):
                tile_piggies.append(set())
            tt += T
            idxcols += T * 8
    TT = tt
    IDXCOLS = idxcols
    call_of = {(c_["grp"], c_["h"]): i for i, c_ in enumerate(calls)}

    # trailing -1 pads are skipped by the gather ucode; per-core valid
    # counts are read at runtime (value_load) into num_idxs_reg
    idxflat = np.full((n_cores, TT * P), -1, np.int16)
    counts = np.ones((n_cores, max(1, len(calls))), np.int32)
    for c in range(n_cores):
        for ci, call in enumerate(calls):
            g, h, T = call["grp"], call["h"], call["T"]
            n = int(cnt3[c, g, h])
            s0 = int(run_start[c, g, h])
            o = call["tstart"] * P
            idxflat[c, o:o + n] = so["idx16"][s0:s0 + n].astype(np.int16)
            if n == 0:  # keep >=1 valid index (ucode/sim requirement)
                idxflat[c, o] = 0
            counts[c, ci] = max(n, 1)
            for t in range(T):
                a, b_ = t * P, min((t + 1) * P, n)
                if a >= n:
                    break
                pres = np.unique(so["pig"][s0 + a:s0 + b_])
                tile_piggies[call["tstart"] + t].update(int(p) for p in pres)

    for t in range(TT):
        if not tile_piggies[t]:
            tile_piggies[t].add(0)

    # matmul ids: one host-built one-hot panel per (tile, pig), grouped by grp
    mmid = np.full((TT, GROUP_NBLK), -1, np.int64)
    grp_mms = [[] for _ in range(NGRP)]  # (ci, t, gt, pig, m)
    m = 0
    for ci, call in enumerate(calls):
        for t in range(call["T"]):
            gt = call["tstart"] + t
            for pig in sorted(tile_piggies[gt]):
                mmid[gt, pig] = m
                grp_mms[call["grp"]].append((ci, t, gt, pig, m))
                m += 1
    NMM = m
    grp_moff = [min((e[4] for e in g_), default=0) for g_ in grp_mms]
    MAXM = max(len(g_) for g_ in grp_mms)

    # wrap-16 + replicate-to-128 index layout, call-local
    gidx = np.zeros((n_cores, P, IDXCOLS), np.int16)
    for call in calls:
        a = call["tstart"] * P
        T = call["T"]
        region = idxflat[:, a:a + T * P]
        arr = region.reshape(n_cores, T * 8, 16).transpose(0, 2, 1)
        gidx[:, :, call["idx_off"]:call["idx_off"] + T * 8] = (
            np.tile(arr, (1, 8, 1)))

    # host-built one-hot scatter panels (shared by both layers)
    bpans = []
    for c in range(n_cores):
        bp = np.zeros((P, NMM * P), BF16)
        for ci, call in enumerate(calls):
            g, h = call["grp"], call["h"]
            n = int(cnt3[c, g, h])
            if n == 0:
                continue
            s0 = int(run_start[c, g, h])
            p_ = np.arange(n)
            gt = call["tstart"] + p_ // P
            sp = p_ % P
            pig_e = so["pig"][s0:s0 + n]
            slot_e = so["slot"][s0:s0 + n]
            m_e = mmid[gt, pig_e]
            bp[sp, m_e * P + slot_e] = 1.0
        bpans.append(bp)

    # --- pooling helpers ---
    rows = np.arange(NP)
    rcore = rows // RPC
    rblk = (rows % RPC) // P
    rslot = rows % P
    pm = np.zeros((n_cores, P, BPC * G), BF16)
    rg = np.where(real, batch[np.clip(row2node, 0, N - 1)], -1)
    val = real
    pm[rcore[val], rslot[val], rblk[val] * G + rg[val]] = 1.0
    pmask = np.zeros((n_cores, P, G * BPC), BF16)
    for c in range(n_cores):
        for b in range(BPC):
            g = g_of_block[c * BPC + b]
            if g >= 0:
                pmask[c, :, g * BPC + b] = 1.0
    recip = (1.0 / np.maximum(cnt, 1.0)).astype(np.float32).reshape(G, 1)
    # last block position (over all cores) of each graph; empty graphs -> 0
    lastpos = np.zeros(G, np.int64)
    for bid in range(total_blocks_padded):
        g = g_of_block[bid]
        if g >= 0:
            lastpos[g] = max(lastpos[g], bid % BPC)
    graphs_at = [[] for _ in range(BPC)]
    for g in range(G):
        graphs_at[int(lastpos[g])].append(g)

    # --- per-core input maps ---
    in_maps = []
    for c in range(n_cores):
        r0, r1 = c * RPC, (c + 1) * RPC
        m = {
            "xt": np.ascontiguousarray(x_pad[r0:r1].T).astype(BF16),
            "w1": np.asarray(W1, np.float32).astype(BF16),
            "w2": np.asarray(W2, np.float32).astype(BF16),
            "wfc": np.asarray(Wfc, np.float32).astype(BF16),
            "b1r": np.asarray(b1, np.float32).reshape(1, FH).astype(BF16),
            "b2r": np.asarray(b2, np.float32).reshape(1, FH).astype(BF16),
            "bfcr": np.asarray(bfc, np.float32).reshape(1, FO).astype(BF16),
            "sqdeg": sqdeg_pad[r0:r1].reshape(1, RPC).astype(BF16),
            "dinv": np.ascontiguousarray(
                dinv_pad[r0:r1].reshape(BPC, P).T).astype(np.float32),
            "gidx": gidx[c],
            "counts": counts[c:c + 1],
            "bpan": bpans[c],
            "pm": pm[c],
            "pmask": pmask[c],
            "recip": recip,
        }
        in_maps.append(m)

    plan = dict(
        G=G, F=F, FH=FH, FO=FO, BPC=BPC, RPC=RPC, NP=NP, HALF=HALF,
        HROWS=HROWS, NGRP=NGRP, TT=TT, IDXCOLS=IDXCOLS, MAXT=MAXT,
        calls=calls, call_of=call_of,
        grp_mms=grp_mms, grp_moff=grp_moff, NMM=NMM, MAXM=MAXM,
        graphs_at=graphs_at,
        n_cores=n_cores,
        has_b1=bool(np.any(np.asarray(b1))),
        has_b2=bool(np.any(np.asarray(b2))),
        has_bfc=bool(np.any(np.asarray(bfc))),
    )
    return plan, in_maps


# --------------------------------------------------------------------------
# Bass program builder (identical on all cores).
# --------------------------------------------------------------------------

def build(plan, debug=False):
    dt = mybir.dt
    G, F, FH, FO = plan["G"], plan["F"], plan["FH"], plan["FO"]
    BPC, RPC, NP = plan["BPC"], plan["RPC"], plan["NP"]
    HROWS, NGRP = plan["HROWS"], plan["NGRP"]
    TT, IDXCOLS, MAXT = plan["TT"], plan["IDXCOLS"], plan["MAXT"]
    calls, call_of = plan["calls"], plan["call_of"]
    grp_mms, grp_moff = plan["grp_mms"], plan["grp_moff"]
    NMM, MAXM = plan["NMM"], plan["MAXM"]
    graphs_at = plan["graphs_at"]
    n_cores = plan["n_cores"]
    KC = F // P          # k-chunks for the transforms (2)
    FCK = (3 * FH) // P  # k-chunks for the FC (6)
    HB = BPC // 2        # blocks per table half

    nc = bacc.Bacc("TRN2", target_bir_lowering=False, debug=debug,
                   num_devices=n_cores)

    def din(name, shape, dtype):
        return nc.dram_tensor(name, shape, dtype, kind="ExternalInput").ap()

    xt_d = din("xt", [F, RPC], dt.bfloat16)
    w1_d = din("w1", [F, FH], dt.bfloat16)
    w2_d = din("w2", [FH, FH], dt.bfloat16)
    wfc_d = din("wfc", [3 * FH, FO], dt.bfloat16)
    b1r_d = din("b1r", [1, FH], dt.bfloat16)
    b2r_d = din("b2r", [1, FH], dt.bfloat16)
    bfcr_d = din("bfcr", [1, FO], dt.bfloat16)
    sqdeg_d = din("sqdeg", [1, RPC], dt.bfloat16)
    dinv_d = din("dinv", [P, BPC], dt.float32)
    gidx_d = din("gidx", [P, IDXCOLS], dt.int16)
    counts_d = din("counts", [1, max(1, len(calls))], dt.int32)
    bpan_d = din("bpan", [P, NMM * P], dt.bfloat16)
    pm_d = din("pm", [P, BPC * G], dt.bfloat16)
    pmask_d = din("pmask", [P, G * BPC], dt.bfloat16)
    recip_d = din("recip", [G, 1], dt.float32)
    out_d = nc.dram_tensor("out", [G, FO], dt.float32,
                           kind="ExternalOutput").ap()

    rg = [list(range(n_cores))]

    from contextlib import ExitStack
    with tile.TileContext(nc) as tc, ExitStack() as ctx:
        const = ctx.enter_context(tc.tile_pool(name="const", bufs=1))
        dram = ctx.enter_context(tc.tile_pool(name="dram", bufs=1, space="DRAM"))
        tfpsum = ctx.enter_context(tc.tile_pool(name="tfpsum", bufs=2, space="PSUM"))
        aggpsum = ctx.enter_context(tc.tile_pool(name="aggpsum", bufs=3, space="PSUM"))
        tpsum = ctx.enter_context(tc.tile_pool(name="tpsum", bufs=1, space="PSUM"))
        spsum = ctx.enter_context(tc.tile_pool(name="spsum", bufs=1, space="PSUM"))
        fcpsum = ctx.enter_context(tc.tile_pool(name="fcpsum", bufs=1, space="PSUM"))
        msgp = ctx.enter_context(tc.tile_pool(name="msgp", bufs=5))
        bpp = ctx.enter_context(tc.tile_pool(name="bpp", bufs=2))
        btp = ctx.enter_context(tc.tile_pool(name="btp", bufs=4))
        hp = ctx.enter_context(tc.tile_pool(name="hp", bufs=3))
        htp = ctx.enter_context(tc.tile_pool(name="htp", bufs=4))
        tailp = ctx.enter_context(tc.tile_pool(name="tailp", bufs=1))

        # ---------------- constants into SBUF ----------------
        def cload(tag, dram_ap, shape, dtype):
            t = const.tile(shape, dtype, tag=tag)
            nc.sync.dma_start(out=t[:], in_=dram_ap)
            return t

        # trigger the first (dummy) collective ASAP: the runtime's one-time
        # collective init (~60us) runs serially before the first real
        # AllGather otherwise
        warm_in = dram.tile([8, 16], dt.bfloat16, tag="warmin")
        warm_out = dram.tile([64, 16], dt.bfloat16, tag="warmout",
                             addr_space="Shared")
        wz = const.tile([8, 16], dt.bfloat16, tag="wz")
        nc.gpsimd.memset(wz[:], 0.0)
        nc.sync.dma_start(out=warm_in[:], in_=wz[:])
        nc.gpsimd.collective_compute(
            "AllGather", mybir.AluOpType.bypass,
            ins=[warm_in[:].opt()], outs=[warm_out[:].opt()],
            replica_groups=rg)

        xt_sb = const.tile([P, KC * RPC], dt.bfloat16, tag="xt")
        for c in range(KC):
            nc.sync.dma_start(out=xt_sb[:, c * RPC:(c + 1) * RPC],
                              in_=xt_d[c * P:(c + 1) * P, :])
        w_sb = []
        for tag, d in (("w1", w1_d), ("w2", w2_d)):
            t = const.tile([P, KC * FH], dt.bfloat16, tag=tag)
            for c in range(KC):
                nc.sync.dma_start(out=t[:, c * FH:(c + 1) * FH],
                                  in_=d[c * P:(c + 1) * P, :])
            w_sb.append(t)
        wfc_sb = const.tile([P, FCK * FO], dt.bfloat16, tag="wfc")
        for c in range(FCK):
            nc.sync.dma_start(out=wfc_sb[:, c * FO:(c + 1) * FO],
                              in_=wfc_d[c * P:(c + 1) * P, :])
        dinv_sb = cload("dinv", dinv_d, [P, BPC], dt.float32)

        iota_sb = const.tile([P, P], dt.float32, tag="iota")
        nc.gpsimd.iota(out=iota_sb[:], pattern=[[1, P]], base=0,
                       channel_multiplier=0,
                       allow_small_or_imprecise_dtypes=True)
        iotac_sb = const.tile([P, 1], dt.float32, tag="iotac")
        nc.gpsimd.iota(out=iotac_sb[:], pattern=[[0, 1]], base=0,
                       channel_multiplier=1,
                       allow_small_or_imprecise_dtypes=True)
        ident_sb = const.tile([P, P], dt.bfloat16, tag="ident")
        nc.vector.tensor_tensor(out=ident_sb[:],
                                in0=iotac_sb[:].to_broadcast([P, P]),
                                in1=iota_sb[:],
                                op=mybir.AluOpType.is_equal)
        ones_sb = const.tile([1, G], dt.bfloat16, tag="ones")
        nc.gpsimd.memset(ones_sb[:], 1.0)
        blockmax = const.tile([P, KC * BPC], dt.bfloat16, tag="bmax")
        nc.gpsimd.memset(blockmax[:], 0.0)
        # persistent per-layer local tables (bf16, dinv-scaled)
        tbl = [const.tile([P, BPC * FH], dt.bfloat16, name=f"tbl{l}",
                          tag=f"tbl{l}")
               for l in range(2)]

        # DRAM bounce buffers for collectives (per layer, per half)
        ag_in = [[dram.tile([HROWS, FH], dt.bfloat16, name=f"agin{l}{h}",
                            tag=f"agin{l}{h}")
                  for h in (0, 1)] for l in range(2)]
        ag_out = [[dram.tile([HROWS * n_cores, FH], dt.bfloat16,
                             name=f"agout{l}{h}", tag=f"agout{l}{h}")
                   for h in (0, 1)] for l in range(2)]
        ars_in = [dram.tile([G, FH], dt.bfloat16, name=f"arsin{k}",
                            tag=f"arsin{k}") for k in (0, 1)]
        ars_out = [dram.tile([G, FH], dt.bfloat16, name=f"arsout{k}",
                             tag=f"arsout{k}", addr_space="Shared")
                   for k in (0, 1)]
        arm_in = dram.tile([P, KC * G], dt.bfloat16, tag="armin")
        arm_out = dram.tile([P, KC * G], dt.bfloat16, tag="armout",
                            addr_space="Shared")

        Copy = mybir.ActivationFunctionType.Copy
        Relu = mybir.ActivationFunctionType.Relu

        cnt_reg = nc.gpsimd.alloc_register("cntreg")

        # zero-fill the msg ring once: slots of skipped (-1 pad) gather
        # indices are never written, and must read as finite for the
        # zero panel rows to nullify them
        for _ in range(5):
            mb0 = msgp.tile([P, MAXT * FH], dt.bfloat16, tag="msg")
            nc.gpsimd.memset(mb0[:], 0.0)

        def allgather(l, h):
            nc.gpsimd.collective_compute(
                "AllGather", mybir.AluOpType.bypass,
                ins=[ag_in[l][h][:].opt()], outs=[ag_out[l][h][:].opt()],
                replica_groups=rg)

        def push_block(l, b):
            h = 0 if b < HB else 1
            r0 = (b - h * HB) * P
            nc.scalar.dma_start(out=ag_in[l][h][r0:r0 + P, :],
                                in_=tbl[l][:, b * FH:(b + 1) * FH])

        # ---------------- layer-1 transform ----------------
        for b in range(BPC):
            ps = tfpsum.tile([P, FH], dt.float32, tag="tfps")
            for c in range(KC):
                nc.tensor.matmul(
                    out=ps[:],
                    lhsT=xt_sb[:, c * RPC + b * P:c * RPC + (b + 1) * P],
                    rhs=w_sb[0][:, c * FH:(c + 1) * FH],
                    start=(c == 0), stop=(c == KC - 1))
            nc.scalar.activation(out=tbl[0][:, b * FH:(b + 1) * FH],
                                 in_=ps[:], func=Copy,
                                 scale=dinv_sb[:, b:b + 1])
            push_block(0, b)
            if b == HB - 1:
                allgather(0, 0)
        allgather(0, 1)

        # deferred const loads (needed from the agg phase on, not by the
        # transforms -- keeps the early sync-DMA queue clear so the first
        # AllGather can fire as soon as the transforms finish)
        gidx_sb = cload("gidx", gidx_d, [P, IDXCOLS], dt.int16)
        counts_sb = cload("counts", counts_d,
                          [1, max(1, len(calls))], dt.int32)
        b1r_sb = cload("b1r", b1r_d, [1, FH], dt.bfloat16)
        b2r_sb = cload("b2r", b2r_d, [1, FH], dt.bfloat16)
        bfcr_sb = cload("bfcr", bfcr_d, [1, FO], dt.bfloat16)
        sqdeg_sb = cload("sqdeg", sqdeg_d, [1, RPC], dt.bfloat16)
        pm_sb = cload("pm", pm_d, [P, BPC * G], dt.bfloat16)
        pmask_sb = cload("pmask", pmask_d, [P, G * BPC], dt.bfloat16)
        recip_sb = cload("recip", recip_d, [G, 1], dt.float32)

        # ---------------- aggregation over edges ----------------
        AWIN = 3  # h=0 calls issued ahead of h=1 calls (hides AG of half 1)

        def agg_layer(l, bias_row, has_bias, produce_block):
            # interleave calls: [g0A g1A g2A g0B g1B ...] with groups'
            # matmul work following once both halves of a group are in.
            mbufs = {}   # call index -> sbuf tile

            def do_call(ci):
                call = calls[ci]
                T = call["T"]
                nc.gpsimd.reg_load(cnt_reg, counts_sb[0:1, ci:ci + 1])
                mb = msgp.tile([P, MAXT * FH], dt.bfloat16, tag="msg")
                out_ap = mb[:, :T * FH].rearrange("p (t e) -> p t e", e=FH)
                nc.gpsimd.dma_gather(
                    out_ap=out_ap,
                    in_ap=ag_out[l][call["h"]][:],
                    idxs_ap=gidx_sb[:, call["idx_off"]:
                                    call["idx_off"] + T * 8],
                    num_idxs=T * P,
                    num_idxs_reg=cnt_reg,
                    elem_size=FH,
                    single_packet=False)
                mbufs[ci] = mb

            order = []   # call issue order (A-window interleave)
            apend = [ci for ci in range(len(calls)) if calls[ci]["h"] == 0]
            bpend = [ci for ci in range(len(calls)) if calls[ci]["h"] == 1]
            ai = bi = 0
            while ai < len(apend) or bi < len(bpend):
                win = AWIN + 1 if bi == 0 else AWIN
                if ai < len(apend) and (ai - bi < win or bi >= len(bpend)):
                    order.append(apend[ai]); ai += 1
                else:
                    order.append(bpend[bi]); bi += 1

            done_upto = 0  # groups fully processed

            def group_ready(g):
                for h in (0, 1):
                    ci = call_of.get((g, h))
                    if ci is not None and ci not in mbufs:
                        return False
                return True

            def process_group(g):
                nmm = len(grp_mms[g])
                moff = grp_moff[g]
                bsl = None
                if nmm:
                    bsl = bpp.tile([P, MAXM * P], dt.bfloat16, tag="bsl")
                    nc.sync.dma_start(
                        out=bsl[:, :nmm * P],
                        in_=bpan_d[:, moff * P:(moff + nmm) * P])
                for pig in range(GROUP_NBLK):
                    b = g * GROUP_NBLK + pig
                    ps = aggpsum.tile([P, FH], dt.float32, tag="aggps")
                    mms = [e for e in grp_mms[g] if e[3] == pig]
                    # self loop: identity matmul against local table block
                    nc.tensor.matmul(out=ps[:], lhsT=ident_sb[:],
                                     rhs=tbl[l][:, b * FH:(b + 1) * FH],
                                     start=True,
                                     stop=(not mms) and not has_bias)
                    for k, (ci, t, gt, _pig, m) in enumerate(mms):
                        mo = m - moff
                        nc.tensor.matmul(
                            out=ps[:], lhsT=bsl[:, mo * P:(mo + 1) * P],
                            rhs=mbufs[ci][:, t * FH:(t + 1) * FH],
                            start=False,
                            stop=(k == len(mms) - 1) and not has_bias)
                    if has_bias:
                        nc.tensor.matmul(
                            out=ps[:],
                            lhsT=sqdeg_sb[:, b * P:(b + 1) * P],
                            rhs=bias_row[:],
                            start=False, stop=True)
                    produce_block(b, ps)

            for ci in order:
                do_call(ci)
                # process any groups that are now complete, in order
                while done_upto < NGRP and group_ready(done_upto):
                    process_group(done_upto)
                    g = done_upto
                    done_upto += 1
                    # release msg buffers of this group
                    for h in (0, 1):
                        cix = call_of.get((g, h))
                        if cix in mbufs:
                            del mbufs[cix]
                    if l == 0 and g == min(NGRP - 1, HB // GROUP_NBLK + 2):
                        allgather(1, 0)
            while done_upto < NGRP:
                process_group(done_upto)
                done_upto += 1
            if l == 0:
                allgather(1, 1)

        # layer-1 block epilogue: relu, transform to layer-2 table
        def produce1(b, ps):
            h1 = hp.tile([P, FH], dt.bfloat16, tag="h1")
            nc.scalar.activation(out=h1[:], in_=ps[:], func=Relu,
                                 scale=dinv_sb[:, b:b + 1])
            h1t = []
            for c in range(KC):
                tp = tpsum.tile([P, P], dt.bfloat16, tag="tp")
                nc.tensor.transpose(out=tp[:],
                                    in_=h1[:, c * P:(c + 1) * P],
                                    identity=ident_sb[:])
                ht = htp.tile([P, P], dt.bfloat16, tag="ht")
                nc.vector.tensor_copy(out=ht[:], in_=tp[:])
                h1t.append(ht)
            ps2 = tfpsum.tile([P, FH], dt.float32, tag="tfps")
            for c in range(KC):
                nc.tensor.matmul(out=ps2[:], lhsT=h1t[c][:],
                                 rhs=w_sb[1][:, c * FH:(c + 1) * FH],
                                 start=(c == 0), stop=(c == KC - 1))
            nc.scalar.activation(out=tbl[1][:, b * FH:(b + 1) * FH],
                                 in_=ps2[:], func=Copy,
                                 scale=dinv_sb[:, b:b + 1])
            push_block(1, b)

        agg_layer(0, b1r_sb, plan["has_b1"], produce1)

        # layer-2 block epilogue: relu, pooling contributions
        sums_ps = spsum.tile([G, FH], dt.float32, tag="sums")
        mxT_loc = const.tile([P, KC * G], dt.bfloat16, tag="mxT_loc")

        SUMS_SPLIT = BPC - 6 if BPC > 8 else -1  # no split on tiny configs

        def produce2(b, ps):
            h2 = hp.tile([P, FH], dt.bfloat16, tag="h2")
            nc.scalar.activation(out=h2[:], in_=ps[:], func=Relu,
                                 scale=dinv_sb[:, b:b + 1])
            nc.tensor.matmul(out=sums_ps[:],
                             lhsT=pm_sb[:, b * G:(b + 1) * G],
                             rhs=h2[:],
                             start=(b == 0 or b == SUMS_SPLIT),
                             stop=(b == SUMS_SPLIT - 1 or b == BPC - 1))
            if b == SUMS_SPLIT - 1:
                ssA = tailp.tile([G, FH], dt.bfloat16, tag="ssA")
                nc.vector.tensor_copy(out=ssA[:], in_=sums_ps[:])
                nc.sync.dma_start(out=ars_in[0][:], in_=ssA[:])
                nc.gpsimd.collective_compute(
                    "AllReduce", mybir.AluOpType.add,
                    ins=[ars_in[0][:].opt()], outs=[ars_out[0][:].opt()],
                    replica_groups=rg)
            for c in range(KC):
                tp = tpsum.tile([P, P], dt.bfloat16, tag="tp")
                nc.tensor.transpose(out=tp[:],
                                    in_=h2[:, c * P:(c + 1) * P],
                                    identity=ident_sb[:])
                nc.vector.tensor_reduce(
                    out=blockmax[:, c * BPC + b:c * BPC + b + 1],
                    in_=tp[:], axis=mybir.AxisListType.X,
                    op=mybir.AluOpType.max)
            # per-graph local max for graphs whose last block is b
            for g in graphs_at[b]:
                mtmp = btp.tile([P, BPC], dt.bfloat16, tag="mtmp")
                for c in range(KC):
                    nc.vector.tensor_tensor(
                        out=mtmp[:],
                        in0=blockmax[:, c * BPC:(c + 1) * BPC],
                        in1=pmask_sb[:, g * BPC:(g + 1) * BPC],
                        op=mybir.AluOpType.mult)
                    nc.vector.tensor_reduce(
                        out=mxT_loc[:, c * G + g:c * G + g + 1],
                        in_=mtmp[:],
                        axis=mybir.AxisListType.X, op=mybir.AluOpType.max)

        agg_layer(1, b2r_sb, plan["has_b2"], produce2)

        # ---------------- pooling tail ----------------
        sums_sb = tailp.tile([G, FH], dt.bfloat16, tag="sums_sb")
        nc.vector.tensor_copy(out=sums_sb[:], in_=sums_ps[:])
        nc.sync.dma_start(out=ars_in[1][:], in_=sums_sb[:])
        nc.gpsimd.collective_compute(
            "AllReduce", mybir.AluOpType.add,
            ins=[ars_in[1][:].opt()], outs=[ars_out[1][:].opt()],
            replica_groups=rg)
        nc.sync.dma_start(out=arm_in[:], in_=mxT_loc[:])
        nc.gpsimd.collective_compute(
            "AllReduce", mybir.AluOpType.max,
            ins=[arm_in[:].opt()], outs=[arm_out[:].opt()],
            replica_groups=rg)

        gsA = tailp.tile([G, FH], dt.bfloat16, tag="gsA")
        if SUMS_SPLIT > 0:
            nc.sync.dma_start(out=gsA[:], in_=ars_out[0][:])
        else:
            nc.gpsimd.memset(gsA[:], 0.0)
        gsB = tailp.tile([G, FH], dt.bfloat16, tag="gsB")
        nc.sync.dma_start(out=gsB[:], in_=ars_out[1][:])
        gsums = tailp.tile([G, FH], dt.bfloat16, tag="gsums")
        nc.vector.tensor_tensor(out=gsums[:], in0=gsA[:], in1=gsB[:],
                                op=mybir.AluOpType.add)
        mxT = tailp.tile([P, KC * G], dt.bfloat16, tag="mxT")
        nc.sync.dma_start(out=mxT[:], in_=arm_out[:])

        # mean / sums in bf16, transposed to feature-major for the FC
        mean_sb = tailp.tile([G, FH], dt.bfloat16, tag="mean")
        nc.vector.tensor_scalar(out=mean_sb[:], in0=gsums[:],
                                scalar1=recip_sb[:], scalar2=None,
                                op0=mybir.AluOpType.mult)
        sums_bf = tailp.tile([G, FH], dt.bfloat16, tag="sumsbf")
        nc.vector.tensor_copy(out=sums_bf[:], in_=gsums[:])
        meanT = tailp.tile([P, KC * G], dt.bfloat16, tag="meanT")
        sumsT = tailp.tile([P, KC * G], dt.bfloat16, tag="sumsT")
        for src, dst_t in ((mean_sb, meanT), (sums_bf, sumsT)):
            for c in range(KC):
                tp = tpsum.tile([P, P], dt.bfloat16, tag="tp")
                nc.tensor.transpose(out=tp[:, :G],
                                    in_=src[:, c * P:(c + 1) * P],
                                    identity=ident_sb[:G, :G])
                nc.vector.tensor_copy(out=dst_t[:, c * G:(c + 1) * G],
                                      in_=tp[:, :G])

        # final FC: out = [mean | max | sums] @ Wfc + bfc
        fc_ps = fcpsum.tile([G, FO], dt.float32, tag="fc")
        gT = [meanT, mxT, sumsT]
        k = 0
        for part in range(3):
            for c in range(KC):
                nc.tensor.matmul(
                    out=fc_ps[:], lhsT=gT[part][:, c * G:(c + 1) * G],
                    rhs=wfc_sb[:, k * FO:(k + 1) * FO],
                    start=(k == 0),
                    stop=(k == FCK - 1) and not plan["has_bfc"])
                k += 1
        if plan["has_bfc"]:
            nc.tensor.matmul(out=fc_ps[:], lhsT=ones_sb[:], rhs=bfcr_sb[:],
                             start=False, stop=True)
        out_sb = tailp.tile([G, FO], dt.float32, tag="out_sb")
        nc.vector.tensor_copy(out=out_sb[:], in_=fc_ps[:])
        nc.sync.dma_start(out=out_d[:], in_=out_sb[:])

    nc.compile()
    return nc


# --------------------------------------------------------------------------
# Entry point for the grading harness.
# --------------------------------------------------------------------------

def kernel(x, edge_index, batch, n_graphs, W1, b1, W2, b2, Wfc, bfc,
           **_unused):
    plan, in_maps = preprocess(x, edge_index, batch, n_graphs,
                               W1, b1, W2, b2, Wfc, bfc)
    nc = build(plan)
    res = run_bass_kernel_spmd(nc, in_maps, core_ids=list(range(NCORES)))
    out = np.asarray(res.results[0]["out"], np.float32)
    return out


# revision 42
# speedup vs baseline: 1.1558x; 1.0300x over previous
"""Trainium2 Bass kernel for a 2-layer GCN (EnhancedGNN) with triple global
pooling and a final FC, run SPMD across 8 NeuronCores.

Strategy:
  - Nodes are re-ordered so every 128-row block belongs to exactly one graph
    ("pure blocks"), padded per-graph to multiples of 128. Blocks are sharded
    contiguously across the 8 cores (dst / data parallel).
  - Per layer: each core transforms its node shard (x @ W, scaled by
    dinv = deg^-1/2) into a bf16 "table" shard kept in SBUF AND pushed to
    DRAM; two AllGathers (one per half of the shard) replicate the table to
    every core so gathers on the first half can start while the second half
    is still in flight.
  - Aggregation: per group of 2 dst blocks and per table half, one
    dma_gather fetches the 512B message rows for all edges of the group
    (edges laid contiguously, slot-coded with a 256*block offset); messages
    are scattered into per-block PSUM accumulators with one-hot matmuls
    (B built on DVE via is_equal against offset iotas). Self-loop terms use
    an identity matmul against the SBUF-resident local table block (no
    gather, and it initializes the PSUM).
  - Pooling: per-graph sums via a one-hot matmul, per-graph max via
    per-block feature-major reduce_max + data-driven graph masks evaluated
    incrementally (as soon as a graph's last block is aggregated), then
    AllReduce(add/max) and a tiny FC run redundantly on every core.

The kernel program is identical on all 8 cores (SPMD); all per-core
differences live in the input data. Structure constants (tile counts etc.)
are maxima over cores so the program is uniform.
"""

import numpy as np
import ml_dtypes

import concourse.bass as bass
import concourse.tile as tile
from concourse import bacc, mybir
from concourse.bass_utils import run_bass_kernel_spmd

P = 128
NCORES = 8
GROUP_NBLK = 2  # dst blocks per gather group

BF16 = ml_dtypes.bfloat16
PAD_SLOT = 1000.0  # slot code that never matches any iota offset


def _cdiv(a, b):
    return -(-a // b)


# --------------------------------------------------------------------------
# Host-side preprocessing: sharding, edge grouping, auxiliary tensors.
# --------------------------------------------------------------------------

def preprocess(x, edge_index, batch, n_graphs, W1, b1, W2, b2, Wfc, bfc,
               n_cores=NCORES):
    x = np.asarray(x, np.float32)
    ei = np.asarray(edge_index, np.int64)
    batch = np.asarray(batch, np.int64)
    G = int(n_graphs)
    N = x.shape[0]
    F = x.shape[1]
    FH = W1.shape[1]
    FO = Wfc.shape[1]
    assert F == FH, "kernel assumes F_IN == F_HID"

    # degrees (dst side, + self loop), as in the reference
    deg = np.bincount(ei[1], minlength=N).astype(np.float32) + 1.0
    dinv = 1.0 / np.sqrt(deg)
    sqdeg = np.sqrt(deg)

    # --- graph-padded node ordering (pure blocks) ---
    cnt = np.bincount(batch, minlength=G).astype(np.int64)  # nodes per graph
    blocks_g = _cdiv(cnt, P)  # 0 for empty graphs
    total_blocks = int(blocks_g.sum())
    # pad so BPC is even (needed for the half-split of each core's shard)
    total_blocks_padded = _cdiv(total_blocks, 2 * n_cores) * 2 * n_cores
    BPC = total_blocks_padded // n_cores
    RPC = BPC * P
    NP = total_blocks_padded * P
    HROWS = RPC // 2            # local rows per table half
    HALF = NP // 2              # rows per table half (all cores)
    assert HALF <= 32768, f"table half {HALF} exceeds int16 index range"
    NGRP = BPC // GROUP_NBLK
    assert BPC % GROUP_NBLK == 0

    blk_start = np.concatenate([[0], np.cumsum(blocks_g)])  # per graph
    row_start = blk_start * P
    first_node = np.concatenate([[0], np.cumsum(cnt)])[:-1]
    new_pos = row_start[batch] + (np.arange(N) - first_node[batch])
    row2node = np.full(NP, -1, np.int64)
    row2node[new_pos] = np.arange(N)
    real = row2node >= 0

    # per padded row data
    x_pad = np.zeros((NP, F), np.float32)
    x_pad[real] = x[row2node[real]]
    dinv_pad = np.ones(NP, np.float32)
    dinv_pad[real] = dinv[row2node[real]]
    sqdeg_pad = np.zeros(NP, np.float32)
    sqdeg_pad[real] = sqdeg[row2node[real]]
    g_of_block = np.full(total_blocks_padded, -1, np.int64)
    for g in range(G):
        g_of_block[blk_start[g]:blk_start[g + 1]] = g

    # --- edges (real edges only; +I loops handled by identity matmuls) ---
    es = new_pos[ei[0]]
    ed = new_pos[ei[1]]
    core = ed // RPC
    pos = (ed % RPC) // P          # block position within core
    grp = pos // GROUP_NBLK
    pig = pos % GROUP_NBLK         # position in group
    slot = ed % P
    lr = es % RPC                  # src local row on its owner core
    half = lr // HROWS
    idx16 = (es // RPC) * HROWS + (lr - half * HROWS)

    # counts per (core, group, half)
    cnt3 = np.zeros((n_cores, NGRP, 2), np.int64)
    np.add.at(cnt3, (core, grp, half), 1)
    Tgh = _cdiv(cnt3.max(axis=0), P)  # [NGRP, 2] tiles, uniform across cores
    MAXT = max(1, int(Tgh.max()))

    # --- per-core edge index / slot arrays, call-ordered ---
    order = np.lexsort((slot, pig, half, grp, core))
    so = dict(core=core[order], grp=grp[order], half=half[order],
              pig=pig[order], slot=slot[order], idx16=idx16[order])
    run_start = np.zeros((n_cores, NGRP, 2), np.int64)
    flat_cnt = cnt3.reshape(-1)
    np.cumsum(flat_cnt[:-1], out=run_start.reshape(-1)[1:])

    # calls in (grp, half) order; record structure
    calls = []      # dicts: grp, h, T, tstart, idx_off
    tile_piggies = []  # per global tile: set of piggies present (union cores)
    tt = 0
    idxcols = 0
    for g in range(NGRP):
        for h in (0, 1):
            T = int(Tgh[g, h])
            if T == 0:
                continue
            calls.append(dict(grp=g, h=h, T=T, tstart=tt, idx_off=idxcols))
            for t in range(T):
                tile_piggies.append(set())
            tt += T
            idxcols += T * 8
    TT = tt
    IDXCOLS = idxcols
    call_of = {(c_["grp"], c_["h"]): i for i, c_ in enumerate(calls)}

    # trailing -1 pads are skipped by the gather ucode; per-core valid
    # counts are read at runtime (value_load) into num_idxs_reg
    idxflat = np.full((n_cores, TT * P), -1, np.int16)
    counts = np.ones((n_cores, max(1, len(calls))), np.int32)
    for c in range(n_cores):
        for ci, call in enumerate(calls):
            g, h, T = call["grp"], call["h"], call["T"]
            n = int(cnt3[c, g, h])
            s0 = int(run_start[c, g, h])
            o = call["tstart"] * P
            idxflat[c, o:o + n] = so["idx16"][s0:s0 + n].astype(np.int16)
            if n == 0:  # keep >=1 valid index (ucode/sim requirement)
                idxflat[c, o] = 0
            counts[c, ci] = max(n, 1)
            for t in range(T):
                a, b_ = t * P, min((t + 1) * P, n)
                if a >= n:
                    break
                pres = np.unique(so["pig"][s0 + a:s0 + b_])
                tile_piggies[call["tstart"] + t].update(int(p) for p in pres)

    for t in range(TT):
        if not tile_piggies[t]:
            tile_piggies[t].add(0)

    # matmul ids: one host-built one-hot panel per (tile, pig), grouped by grp
    mmid = np.full((TT, GROUP_NBLK), -1, np.int64)
    grp_mms = [[] for _ in range(NGRP)]  # (ci, t, gt, pig, m)
    m = 0
    for ci, call in enumerate(calls):
        for t in range(call["T"]):
            gt = call["tstart"] + t
            for pig in sorted(tile_piggies[gt]):
                mmid[gt, pig] = m
                grp_mms[call["grp"]].append((ci, t, gt, pig, m))
                m += 1
    NMM = m
    grp_moff = [min((e[4] for e in g_), default=0) for g_ in grp_mms]
    MAXM = max(len(g_) for g_ in grp_mms)

    # wrap-16 + replicate-to-128 index layout, call-local
    gidx = np.zeros((n_cores, P, IDXCOLS), np.int16)
    for call in calls:
        a = call["tstart"] * P
        T = call["T"]
        region = idxflat[:, a:a + T * P]
        arr = region.reshape(n_cores, T * 8, 16).transpose(0, 2, 1)
        gidx[:, :, call["idx_off"]:call["idx_off"] + T * 8] = (
            np.tile(arr, (1, 8, 1)))

    # host-built one-hot scatter panels (shared by both layers)
    bpans = []
    for c in range(n_cores):
        bp = np.zeros((P, NMM * P), BF16)
        for ci, call in enumerate(calls):
            g, h = call["grp"], call["h"]
            n = int(cnt3[c, g, h])
            if n == 0:
                continue
            s0 = int(run_start[c, g, h])
            p_ = np.arange(n)
            gt = call["tstart"] + p_ // P
            sp = p_ % P
            pig_e = so["pig"][s0:s0 + n]
            slot_e = so["slot"][s0:s0 + n]
            m_e = mmid[gt, pig_e]
            bp[sp, m_e * P + slot_e] = 1.0
        bpans.append(bp)

    # --- pooling helpers ---
    rows = np.arange(NP)
    rcore = rows // RPC
    rblk = (rows % RPC) // P
    rslot = rows % P
    pm = np.zeros((n_cores, P, BPC * G), BF16)
    rg = np.where(real, batch[np.clip(row2node, 0, N - 1)], -1)
    val = real
    pm[rcore[val], rslot[val], rblk[val] * G + rg[val]] = 1.0
    pmask = np.zeros((n_cores, P, G * BPC), BF16)
    for c in range(n_cores):
        for b in range(BPC):
            g = g_of_block[c * BPC + b]
            if g >= 0:
                pmask[c, :, g * BPC + b] = 1.0
    recip = (1.0 / np.maximum(cnt, 1.0)).astype(np.float32).reshape(G, 1)
    # last block position (over all cores) of each graph; empty graphs -> 0
    lastpos = np.zeros(G, np.int64)
    for bid in range(total_blocks_padded):
        g = g_of_block[bid]
        if g >= 0:
            lastpos[g] = max(lastpos[g], bid % BPC)
    graphs_at = [[] for _ in range(BPC)]
    for g in range(G):
        graphs_at[int(lastpos[g])].append(g)

    # --- per-core input maps ---
    in_maps = []
    for c in range(n_cores):
        r0, r1 = c * RPC, (c + 1) * RPC
        m = {
            "xt": np.ascontiguousarray(x_pad[r0:r1].T).astype(BF16),
            "w1": np.asarray(W1, np.float32).astype(BF16),
            "w2": np.asarray(W2, np.float32).astype(BF16),
            "wfc": np.asarray(Wfc, np.float32).astype(BF16),
            "b1r": np.asarray(b1, np.float32).reshape(1, FH).astype(BF16),
            "b2r": np.asarray(b2, np.float32).reshape(1, FH).astype(BF16),
            "bfcr": np.asarray(bfc, np.float32).reshape(1, FO).astype(BF16),
            "sqdeg": sqdeg_pad[r0:r1].reshape(1, RPC).astype(BF16),
            "dinv": np.ascontiguousarray(
                dinv_pad[r0:r1].reshape(BPC, P).T).astype(np.float32),
            "gidx": gidx[c],
            "counts": counts[c:c + 1],
            "bpan": bpans[c],
            "pm": pm[c],
            "pmask": pmask[c],
            "recip": recip,
        }
        in_maps.append(m)

    plan = dict(
        G=G, F=F, FH=FH, FO=FO, BPC=BPC, RPC=RPC, NP=NP, HALF=HALF,
        HROWS=HROWS, NGRP=NGRP, TT=TT, IDXCOLS=IDXCOLS, MAXT=MAXT,
        calls=calls, call_of=call_of,
        grp_mms=grp_mms, grp_moff=grp_moff, NMM=NMM, MAXM=MAXM,
        graphs_at=graphs_at,
        n_cores=n_cores,
        has_b1=bool(np.any(np.asarray(b1))),
        has_b2=bool(np.any(np.asarray(b2))),
        has_bfc=bool(np.any(np.asarray(bfc))),
    )
    return plan, in_maps


# --------------------------------------------------------------------------
# Bass program builder (identical on all cores).
# --------------------------------------------------------------------------

def build(plan, debug=False):
    dt = mybir.dt
    G, F, FH, FO = plan["G"], plan["F"], plan["FH"], plan["FO"]
    BPC, RPC, NP = plan["BPC"], plan["RPC"], plan["NP"]
    HROWS, NGRP = plan["HROWS"], plan["NGRP"]
    TT, IDXCOLS, MAXT = plan["TT"], plan["IDXCOLS"], plan["MAXT"]
    calls, call_of = plan["calls"], plan["call_of"]
    grp_mms, grp_moff = plan["grp_mms"], plan["grp_moff"]
    NMM, MAXM = plan["NMM"], plan["MAXM"]
    graphs_at = plan["graphs_at"]
    n_cores = plan["n_cores"]
    KC = F // P          # k-chunks for the transforms (2)
    FCK = (3 * FH) // P  # k-chunks for the FC (6)
    HB = BPC // 2        # blocks per table half

    nc = bacc.Bacc("TRN2", target_bir_lowering=False, debug=debug,
                   num_devices=n_cores)

    def din(name, shape, dtype):
        return nc.dram_tensor(name, shape, dtype, kind="ExternalInput").ap()

    xt_d = din("xt", [F, RPC], dt.bfloat16)
    w1_d = din("w1", [F, FH], dt.bfloat16)
    w2_d = din("w2", [FH, FH], dt.bfloat16)
    wfc_d = din("wfc", [3 * FH, FO], dt.bfloat16)
    b1r_d = din("b1r", [1, FH], dt.bfloat16)
    b2r_d = din("b2r", [1, FH], dt.bfloat16)
    bfcr_d = din("bfcr", [1, FO], dt.bfloat16)
    sqdeg_d = din("sqdeg", [1, RPC], dt.bfloat16)
    dinv_d = din("dinv", [P, BPC], dt.float32)
    gidx_d = din("gidx", [P, IDXCOLS], dt.int16)
    counts_d = din("counts", [1, max(1, len(calls))], dt.int32)
    bpan_d = din("bpan", [P, NMM * P], dt.bfloat16)
    pm_d = din("pm", [P, BPC * G], dt.bfloat16)
    pmask_d = din("pmask", [P, G * BPC], dt.bfloat16)
    recip_d = din("recip", [G, 1], dt.float32)
    out_d = nc.dram_tensor("out", [G, FO], dt.float32,
                           kind="ExternalOutput").ap()

    rg = [list(range(n_cores))]

    from contextlib import ExitStack
    with tile.TileContext(nc) as tc, ExitStack() as ctx:
        const = ctx.enter_context(tc.tile_pool(name="const", bufs=1))
        dram = ctx.enter_context(tc.tile_pool(name="dram", bufs=1, space="DRAM"))
        tfpsum = ctx.enter_context(tc.tile_pool(name="tfpsum", bufs=2, space="PSUM"))
        aggpsum = ctx.enter_context(tc.tile_pool(name="aggpsum", bufs=3, space="PSUM"))
        tpsum = ctx.enter_context(tc.tile_pool(name="tpsum", bufs=1, space="PSUM"))
        spsum = ctx.enter_context(tc.tile_pool(name="spsum", bufs=1, space="PSUM"))
        fcpsum = ctx.enter_context(tc.tile_pool(name="fcpsum", bufs=1, space="PSUM"))
        msgp = ctx.enter_context(tc.tile_pool(name="msgp", bufs=5))
        bpp = ctx.enter_context(tc.tile_pool(name="bpp", bufs=2))
        btp = ctx.enter_context(tc.tile_pool(name="btp", bufs=4))
        hp = ctx.enter_context(tc.tile_pool(name="hp", bufs=3))
        htp = ctx.enter_context(tc.tile_pool(name="htp", bufs=4))
        tailp = ctx.enter_context(tc.tile_pool(name="tailp", bufs=1))

        # ---------------- constants into SBUF ----------------
        def cload(tag, dram_ap, shape, dtype):
            t = const.tile(shape, dtype, tag=tag)
            nc.sync.dma_start(out=t[:], in_=dram_ap)
            return t

        # trigger the first (dummy) collective ASAP: the runtime's one-time
        # collective init (~60us) runs serially before the first real
        # AllGather otherwise
        warm_in = dram.tile([8, 16], dt.bfloat16, tag="warmin")
        warm_out = dram.tile([64, 16], dt.bfloat16, tag="warmout",
                             addr_space="Shared")
        wz = const.tile([8, 16], dt.bfloat16, tag="wz")
        nc.gpsimd.memset(wz[:], 0.0)
        nc.sync.dma_start(out=warm_in[:], in_=wz[:])
        nc.gpsimd.collective_compute(
            "AllGather", mybir.AluOpType.bypass,
            ins=[warm_in[:].opt()], outs=[warm_out[:].opt()],
            replica_groups=rg)

        gidx_sb = const.tile([P, IDXCOLS], dt.int16, tag="gidx")
        counts_sb = const.tile([1, max(1, len(calls))], dt.int32,
                               tag="counts")
        xt_sb = const.tile([P, KC * RPC], dt.bfloat16, tag="xt")
        for c in range(KC):
            nc.sync.dma_start(out=xt_sb[:, c * RPC:(c + 1) * RPC],
                              in_=xt_d[c * P:(c + 1) * P, :])
        w_sb = []
        for tag, d in (("w1", w1_d), ("w2", w2_d)):
            t = const.tile([P, KC * FH], dt.bfloat16, tag=tag)
            for c in range(KC):
                nc.sync.dma_start(out=t[:, c * FH:(c + 1) * FH],
                                  in_=d[c * P:(c + 1) * P, :])
            w_sb.append(t)
        wfc_sb = const.tile([P, FCK * FO], dt.bfloat16, tag="wfc")
        for c in range(FCK):
            nc.sync.dma_start(out=wfc_sb[:, c * FO:(c + 1) * FO],
                              in_=wfc_d[c * P:(c + 1) * P, :])
        dinv_sb = cload("dinv", dinv_d, [P, BPC], dt.float32)

        iota_sb = const.tile([P, P], dt.float32, tag="iota")
        nc.gpsimd.iota(out=iota_sb[:], pattern=[[1, P]], base=0,
                       channel_multiplier=0,
                       allow_small_or_imprecise_dtypes=True)
        iotac_sb = const.tile([P, 1], dt.float32, tag="iotac")
        nc.gpsimd.iota(out=iotac_sb[:], pattern=[[0, 1]], base=0,
                       channel_multiplier=1,
                       allow_small_or_imprecise_dtypes=True)
        ident_sb = const.tile([P, P], dt.bfloat16, tag="ident")
        nc.vector.tensor_tensor(out=ident_sb[:],
                                in0=iotac_sb[:].to_broadcast([P, P]),
                                in1=iota_sb[:],
                                op=mybir.AluOpType.is_equal)
        ones_sb = const.tile([1, G], dt.bfloat16, tag="ones")
        nc.gpsimd.memset(ones_sb[:], 1.0)
        blockmax = const.tile([P, KC * BPC], dt.bfloat16, tag="bmax")
        nc.gpsimd.memset(blockmax[:], 0.0)
        # persistent per-layer local tables (bf16, dinv-scaled)
        tbl = [const.tile([P, BPC * FH], dt.bfloat16, name=f"tbl{l}",
                          tag=f"tbl{l}")
               for l in range(2)]

        # DRAM bounce buffers for collectives (per layer, per half)
        ag_in = [[dram.tile([HROWS, FH], dt.bfloat16, name=f"agin{l}{h}",
                            tag=f"agin{l}{h}")
                  for h in (0, 1)] for l in range(2)]
        ag_out = [[dram.tile([HROWS * n_cores, FH], dt.bfloat16,
                             name=f"agout{l}{h}", tag=f"agout{l}{h}")
                   for h in (0, 1)] for l in range(2)]
        ars_in = [dram.tile([G, FH], dt.bfloat16, name=f"arsin{k}",
                            tag=f"arsin{k}") for k in (0, 1)]
        ars_out = [dram.tile([G, FH], dt.bfloat16, name=f"arsout{k}",
                             tag=f"arsout{k}", addr_space="Shared")
                   for k in (0, 1)]
        arm_in = dram.tile([P, KC * G], dt.bfloat16, tag="armin")
        arm_out = dram.tile([P, KC * G], dt.bfloat16, tag="armout",
                            addr_space="Shared")

        Copy = mybir.ActivationFunctionType.Copy
        Relu = mybir.ActivationFunctionType.Relu

        cnt_reg = nc.gpsimd.alloc_register("cntreg")

        # zero-fill the msg ring once: slots of skipped (-1 pad) gather
        # indices are never written, and must read as finite for the
        # zero panel rows to nullify them
        for _ in range(5):
            mb0 = msgp.tile([P, MAXT * FH], dt.bfloat16, tag="msg")
            nc.gpsimd.memset(mb0[:], 0.0)

        def allgather(l, h):
            nc.gpsimd.collective_compute(
                "AllGather", mybir.AluOpType.bypass,
                ins=[ag_in[l][h][:].opt()], outs=[ag_out[l][h][:].opt()],
                replica_groups=rg)

        def push_block(l, b):
            h = 0 if b < HB else 1
            r0 = (b - h * HB) * P
            nc.scalar.dma_start(out=ag_in[l][h][r0:r0 + P, :],
                                in_=tbl[l][:, b * FH:(b + 1) * FH])

        # ---------------- layer-1 transform ----------------
        for b in range(BPC):
            ps = tfpsum.tile([P, FH], dt.float32, tag="tfps")
            for c in range(KC):
                nc.tensor.matmul(
                    out=ps[:],
                    lhsT=xt_sb[:, c * RPC + b * P:c * RPC + (b + 1) * P],
                    rhs=w_sb[0][:, c * FH:(c + 1) * FH],
                    start=(c == 0), stop=(c == KC - 1))
            nc.scalar.activation(out=tbl[0][:, b * FH:(b + 1) * FH],
                                 in_=ps[:], func=Copy,
                                 scale=dinv_sb[:, b:b + 1])
            push_block(0, b)
            if b == HB - 1:
                allgather(0, 0)
        allgather(0, 1)

        # deferred const loads (needed from the agg phase on, not by the
        # transforms -- keeps the early sync-DMA queue clear so the first
        # AllGather can fire as soon as the transforms finish)
        nc.sync.dma_start(out=gidx_sb[:], in_=gidx_d)
        nc.sync.dma_start(out=counts_sb[:], in_=counts_d)
        b1r_sb = cload("b1r", b1r_d, [1, FH], dt.bfloat16)
        b2r_sb = cload("b2r", b2r_d, [1, FH], dt.bfloat16)
        bfcr_sb = cload("bfcr", bfcr_d, [1, FO], dt.bfloat16)
        sqdeg_sb = cload("sqdeg", sqdeg_d, [1, RPC], dt.bfloat16)
        pm_sb = cload("pm", pm_d, [P, BPC * G], dt.bfloat16)
        pmask_sb = cload("pmask", pmask_d, [P, G * BPC], dt.bfloat16)
        recip_sb = cload("recip", recip_d, [G, 1], dt.float32)

        # ---------------- aggregation over edges ----------------
        AWIN = 3  # h=0 calls issued ahead of h=1 calls (hides AG of half 1)

        def agg_layer(l, bias_row, has_bias, produce_block):
            # interleave calls: [g0A g1A g2A g0B g1B ...] with groups'
            # matmul work following once both halves of a group are in.
            mbufs = {}   # call index -> sbuf tile

            def do_call(ci):
                call = calls[ci]
                T = call["T"]
                nc.gpsimd.reg_load(cnt_reg, counts_sb[0:1, ci:ci + 1])
                mb = msgp.tile([P, MAXT * FH], dt.bfloat16, tag="msg")
                out_ap = mb[:, :T * FH].rearrange("p (t e) -> p t e", e=FH)
                nc.gpsimd.dma_gather(
                    out_ap=out_ap,
                    in_ap=ag_out[l][call["h"]][:],
                    idxs_ap=gidx_sb[:, call["idx_off"]:
                                    call["idx_off"] + T * 8],
                    num_idxs=T * P,
                    num_idxs_reg=cnt_reg,
                    elem_size=FH,
                    single_packet=False)
                mbufs[ci] = mb

            order = []   # call issue order (A-window interleave)
            apend = [ci for ci in range(len(calls)) if calls[ci]["h"] == 0]
            bpend = [ci for ci in range(len(calls)) if calls[ci]["h"] == 1]
            ai = bi = 0
            while ai < len(apend) or bi < len(bpend):
                win = AWIN + 1 if bi == 0 else AWIN
                if ai < len(apend) and (ai - bi < win or bi >= len(bpend)):
                    order.append(apend[ai]); ai += 1
                else:
                    order.append(bpend[bi]); bi += 1

            done_upto = 0  # groups fully processed

            def group_ready(g):
                for h in (0, 1):
                    ci = call_of.get((g, h))
                    if ci is not None and ci not in mbufs:
                        return False
                return True

            def process_group(g):
                nmm = len(grp_mms[g])
                moff = grp_moff[g]
                bsl = None
                if nmm:
                    bsl = bpp.tile([P, MAXM * P], dt.bfloat16, tag="bsl")
                    nc.sync.dma_start(
                        out=bsl[:, :nmm * P],
                        in_=bpan_d[:, moff * P:(moff + nmm) * P])
                for pig in range(GROUP_NBLK):
                    b = g * GROUP_NBLK + pig
                    ps = aggpsum.tile([P, FH], dt.float32, tag="aggps")
                    mms = [e for e in grp_mms[g] if e[3] == pig]
                    # self loop: identity matmul against local table block
                    nc.tensor.matmul(out=ps[:], lhsT=ident_sb[:],
                                     rhs=tbl[l][:, b * FH:(b + 1) * FH],
                                     start=True,
                                     stop=(not mms) and not has_bias)
                    for k, (ci, t, gt, _pig, m) in enumerate(mms):
                        mo = m - moff
                        nc.tensor.matmul(
                            out=ps[:], lhsT=bsl[:, mo * P:(mo + 1) * P],
                            rhs=mbufs[ci][:, t * FH:(t + 1) * FH],
                            start=False,
                            stop=(k == len(mms) - 1) and not has_bias)
                    if has_bias:
                        nc.tensor.matmul(
                            out=ps[:],
                            lhsT=sqdeg_sb[:, b * P:(b + 1) * P],
                            rhs=bias_row[:],
                            start=False, stop=True)
                    produce_block(b, ps)

            for ci in order:
                do_call(ci)
                # process any groups that are now complete, in order
                while done_upto < NGRP and group_ready(done_upto):
                    process_group(done_upto)
                    g = done_upto
                    done_upto += 1
                    # release msg buffers of this group
                    for h in (0, 1):
                        cix = call_of.get((g, h))
                        if cix in mbufs:
                            del mbufs[cix]
                    if l == 0 and g == min(NGRP - 1, HB // GROUP_NBLK + 2):
                        allgather(1, 0)
            while done_upto < NGRP:
                process_group(done_upto)
                done_upto += 1
            if l == 0:
                allgather(1, 1)

        # layer-1 block epilogue: relu, transform to layer-2 table
        def produce1(b, ps):
            h1 = hp.tile([P, FH], dt.bfloat16, tag="h1")
            nc.scalar.activation(out=h1[:], in_=ps[:], func=Relu,
                                 scale=dinv_sb[:, b:b + 1])
            h1t = []
            for c in range(KC):
                tp = tpsum.tile([P, P], dt.bfloat16, tag="tp")
                nc.tensor.transpose(out=tp[:],
                                    in_=h1[:, c * P:(c + 1) * P],
                                    identity=ident_sb[:])
                ht = htp.tile([P, P], dt.bfloat16, tag="ht")
                nc.vector.tensor_copy(out=ht[:], in_=tp[:])
                h1t.append(ht)
            ps2 = tfpsum.tile([P, FH], dt.float32, tag="tfps")
            for c in range(KC):
                nc.tensor.matmul(out=ps2[:], lhsT=h1t[c][:],
                                 rhs=w_sb[1][:, c * FH:(c + 1) * FH],
                                 start=(c == 0), stop=(c == KC - 1))
            nc.scalar.activation(out=tbl[1][:, b * FH:(b + 1) * FH],
                                 in_=ps2[:], func=Copy,
                                 scale=dinv_sb[:, b:b + 1])
            push_block(1, b)

        agg_layer(0, b1r_sb, plan["has_b1"], produce1)

        # layer-2 block epilogue: relu, pooling contributions
        sums_ps = spsum.tile([G, FH], dt.float32, tag="sums")
        mxT_loc = const.tile([P, KC * G], dt.bfloat16, tag="mxT_loc")

        SUMS_SPLIT = BPC - 6 if BPC > 8 else -1  # no split on tiny configs

        def produce2(b, ps):
            h2 = hp.tile([P, FH], dt.bfloat16, tag="h2")
            nc.scalar.activation(out=h2[:], in_=ps[:], func=Relu,
                                 scale=dinv_sb[:, b:b + 1])
            nc.tensor.matmul(out=sums_ps[:],
                             lhsT=pm_sb[:, b * G:(b + 1) * G],
                             rhs=h2[:],
                             start=(b == 0 or b == SUMS_SPLIT),
                             stop=(b == SUMS_SPLIT - 1 or b == BPC - 1))
            if b == SUMS_SPLIT - 1:
                ssA = tailp.tile([G, FH], dt.bfloat16, tag="ssA")
                nc.vector.tensor_copy(out=ssA[:], in_=sums_ps[:])
                nc.sync.dma_start(out=ars_in[0][:], in_=ssA[:])
                nc.gpsimd.collective_compute(
                    "AllReduce", mybir.AluOpType.add,
                    ins=[ars_in[0][:].opt()], outs=[ars_out[0][:].opt()],
                    replica_groups=rg)
            for c in range(KC):
                tp = tpsum.tile([P, P], dt.bfloat16, tag="tp")
                nc.tensor.transpose(out=tp[:],
                                    in_=h2[:, c * P:(c + 1) * P],
                                    identity=ident_sb[:])
                nc.vector.tensor_reduce(
                    out=blockmax[:, c * BPC + b:c * BPC + b + 1],
                    in_=tp[:], axis=mybir.AxisListType.X,
                    op=mybir.AluOpType.max)
            # per-graph local max for graphs whose last block is b
            for g in graphs_at[b]:
                mtmp = btp.tile([P, BPC], dt.bfloat16, tag="mtmp")
                for c in range(KC):
                    nc.vector.tensor_tensor(
                        out=mtmp[:],
                        in0=blockmax[:, c * BPC:(c + 1) * BPC],
                        in1=pmask_sb[:, g * BPC:(g + 1) * BPC],
                        op=mybir.AluOpType.mult)
                    nc.vector.tensor_reduce(
                        out=mxT_loc[:, c * G + g:c * G + g + 1],
                        in_=mtmp[:],
                        axis=mybir.AxisListType.X, op=mybir.AluOpType.max)

        agg_layer(1, b2r_sb, plan["has_b2"], produce2)

        # ---------------- pooling tail ----------------
        sums_sb = tailp.tile([G, FH], dt.bfloat16, tag="sums_sb")
        nc.vector.tensor_copy(out=sums_sb[:], in_=sums_ps[:])
        nc.sync.dma_start(out=ars_in[1][:], in_=sums_sb[:])
        nc.gpsimd.collective_compute(
            "AllReduce", mybir.AluOpType.add,
            ins=[ars_in[1][:].opt()], outs=[ars_out[1][:].opt()],
            replica_groups=rg)
        nc.sync.dma_start(out=arm_in[:], in_=mxT_loc[:])
        nc.gpsimd.collective_compute(
            "AllReduce", mybir.AluOpType.max,
            ins=[arm_in[:].opt()], outs=[arm_out[:].opt()],
            replica_groups=rg)

        gsA = tailp.tile([G, FH], dt.bfloat16, tag="gsA")
        if SUMS_SPLIT > 0:
            nc.sync.dma_start(out=gsA[:], in_=ars_out[0][:])
        else:
            nc.gpsimd.memset(gsA[:], 0.0)
        gsB = tailp.tile([G, FH], dt.bfloat16, tag="gsB")
        nc.sync.dma_start(out=gsB[:], in_=ars_out[1][:])
        gsums = tailp.tile([G, FH], dt.bfloat16, tag="gsums")
        nc.vector.tensor_tensor(out=gsums[:], in0=gsA[:], in1=gsB[:],
                                op=mybir.AluOpType.add)
        mxT = tailp.tile([P, KC * G], dt.bfloat16, tag="mxT")
        nc.sync.dma_start(out=mxT[:], in_=arm_out[:])

        # mean / sums in bf16, transposed to feature-major for the FC
        mean_sb = tailp.tile([G, FH], dt.bfloat16, tag="mean")
        nc.vector.tensor_scalar(out=mean_sb[:], in0=gsums[:],
                                scalar1=recip_sb[:], scalar2=None,
                                op0=mybir.AluOpType.mult)
        sums_bf = tailp.tile([G, FH], dt.bfloat16, tag="sumsbf")
        nc.vector.tensor_copy(out=sums_bf[:], in_=gsums[:])
        meanT = tailp.tile([P, KC * G], dt.bfloat16, tag="meanT")
        sumsT = tailp.tile([P, KC * G], dt.bfloat16, tag="sumsT")
        for src, dst_t in ((mean_sb, meanT), (sums_bf, sumsT)):
            for c in range(KC):
                tp = tpsum.tile([P, P], dt.bfloat16, tag="tp")
                nc.tensor.transpose(out=tp[:, :G],
                                    in_=src[:, c * P:(c + 1) * P],
                                    identity=ident_sb[:G, :G])
                nc.vector.tensor_copy(out=dst_t[:, c * G:(c + 1) * G],
                                      in_=tp[:, :G])

        # final FC: out = [mean | max | sums] @ Wfc + bfc
        fc_ps = fcpsum.tile([G, FO], dt.float32, tag="fc")
        gT = [meanT, mxT, sumsT]
        k = 0
        for part in range(3):
            for c in range(KC):
                nc.tensor.matmul(
                    out=fc_ps[:], lhsT=gT[part][:, c * G:(c + 1) * G],
                    rhs=wfc_sb[:, k * FO:(k + 1) * FO],
                    start=(k == 0),
                    stop=(k == FCK - 1) and not plan["has_bfc"])
                k += 1
        if plan["has_bfc"]:
            nc.tensor.matmul(out=fc_ps[:], lhsT=ones_sb[:], rhs=bfcr_sb[:],
                             start=False, stop=True)
        out_sb = tailp.tile([G, FO], dt.float32, tag="out_sb")
        nc.vector.tensor_copy(out=out_sb[:], in_=fc_ps[:])
        nc.sync.dma_start(out=out_d[:], in_=out_sb[:])

    nc.compile()
    return nc


# --------------------------------------------------------------------------
# Entry point for the grading harness.
# --------------------------------------------------------------------------

def kernel(x, edge_index, batch, n_graphs, W1, b1, W2, b2, Wfc, bfc,
           **_unused):
    plan, in_maps = preprocess(x, edge_index, batch, n_graphs,
                               W1, b1, W2, b2, Wfc, bfc)
    nc = build(plan)
    res = run_bass_kernel_spmd(nc, in_maps, core_ids=list(range(NCORES)))
    out = np.asarray(res.results[0]["out"], np.float32)
    return out


# revision 43
# speedup vs baseline: 1.1603x; 1.0038x over previous
"""Trainium2 Bass kernel for a 2-layer GCN (EnhancedGNN) with triple global
pooling and a final FC, run SPMD across 8 NeuronCores.

Strategy:
  - Nodes are re-ordered so every 128-row block belongs to exactly one graph
    ("pure blocks"), padded per-graph to multiples of 128. Blocks are sharded
    contiguously across the 8 cores (dst / data parallel).
  - Per layer: each core transforms its node shard (x @ W, scaled by
    dinv = deg^-1/2) into a bf16 "table" shard kept in SBUF AND pushed to
    DRAM; two AllGathers (one per half of the shard) replicate the table to
    every core so gathers on the first half can start while the second half
    is still in flight.
  - Aggregation: per group of 2 dst blocks and per table half, one
    dma_gather fetches the 512B message rows for all edges of the group
    (edges laid contiguously, slot-coded with a 256*block offset); messages
    are scattered into per-block PSUM accumulators with one-hot matmuls
    (B built on DVE via is_equal against offset iotas). Self-loop terms use
    an identity matmul against the SBUF-resident local table block (no
    gather, and it initializes the PSUM).
  - Pooling: per-graph sums via a one-hot matmul, per-graph max via
    per-block feature-major reduce_max + data-driven graph masks evaluated
    incrementally (as soon as a graph's last block is aggregated), then
    AllReduce(add/max) and a tiny FC run redundantly on every core.

The kernel program is identical on all 8 cores (SPMD); all per-core
differences live in the input data. Structure constants (tile counts etc.)
are maxima over cores so the program is uniform.
"""

import numpy as np
import ml_dtypes

import concourse.bass as bass
import concourse.tile as tile
from concourse import bacc, mybir
from concourse.bass_utils import run_bass_kernel_spmd

P = 128
NCORES = 8
GROUP_NBLK = 2  # dst blocks per gather group

BF16 = ml_dtypes.bfloat16
PAD_SLOT = 1000.0  # slot code that never matches any iota offset


def _cdiv(a, b):
    return -(-a // b)


# --------------------------------------------------------------------------
# Host-side preprocessing: sharding, edge grouping, auxiliary tensors.
# --------------------------------------------------------------------------

def preprocess(x, edge_index, batch, n_graphs, W1, b1, W2, b2, Wfc, bfc,
               n_cores=NCORES):
    x = np.asarray(x, np.float32)
    ei = np.asarray(edge_index, np.int64)
    batch = np.asarray(batch, np.int64)
    G = int(n_graphs)
    N = x.shape[0]
    F = x.shape[1]
    FH = W1.shape[1]
    FO = Wfc.shape[1]
    assert F == FH, "kernel assumes F_IN == F_HID"

    # degrees (dst side, + self loop), as in the reference
    deg = np.bincount(ei[1], minlength=N).astype(np.float32) + 1.0
    dinv = 1.0 / np.sqrt(deg)
    sqdeg = np.sqrt(deg)

    # --- graph-padded node ordering (pure blocks) ---
    cnt = np.bincount(batch, minlength=G).astype(np.int64)  # nodes per graph
    blocks_g = _cdiv(cnt, P)  # 0 for empty graphs
    total_blocks = int(blocks_g.sum())
    # pad so BPC is even (needed for the half-split of each core's shard)
    total_blocks_padded = _cdiv(total_blocks, 2 * n_cores) * 2 * n_cores
    BPC = total_blocks_padded // n_cores
    RPC = BPC * P
    NP = total_blocks_padded * P
    HROWS = RPC // 2            # local rows per table half
    HALF = NP // 2              # rows per table half (all cores)
    assert HALF <= 32768, f"table half {HALF} exceeds int16 index range"
    NGRP = BPC // GROUP_NBLK
    assert BPC % GROUP_NBLK == 0

    blk_start = np.concatenate([[0], np.cumsum(blocks_g)])  # per graph
    row_start = blk_start * P
    first_node = np.concatenate([[0], np.cumsum(cnt)])[:-1]
    new_pos = row_start[batch] + (np.arange(N) - first_node[batch])
    row2node = np.full(NP, -1, np.int64)
    row2node[new_pos] = np.arange(N)
    real = row2node >= 0

    # per padded row data
    x_pad = np.zeros((NP, F), np.float32)
    x_pad[real] = x[row2node[real]]
    dinv_pad = np.ones(NP, np.float32)
    dinv_pad[real] = dinv[row2node[real]]
    sqdeg_pad = np.zeros(NP, np.float32)
    sqdeg_pad[real] = sqdeg[row2node[real]]
    g_of_block = np.full(total_blocks_padded, -1, np.int64)
    for g in range(G):
        g_of_block[blk_start[g]:blk_start[g + 1]] = g

    # --- edges (real edges only; +I loops handled by identity matmuls) ---
    es = new_pos[ei[0]]
    ed = new_pos[ei[1]]
    core = ed // RPC
    pos = (ed % RPC) // P          # block position within core
    grp = pos // GROUP_NBLK
    pig = pos % GROUP_NBLK         # position in group
    slot = ed % P
    lr = es % RPC                  # src local row on its owner core
    half = lr // HROWS
    idx16 = (es // RPC) * HROWS + (lr - half * HROWS)

    # counts per (core, group, half)
    cnt3 = np.zeros((n_cores, NGRP, 2), np.int64)
    np.add.at(cnt3, (core, grp, half), 1)
    Tgh = _cdiv(cnt3.max(axis=0), P)  # [NGRP, 2] tiles, uniform across cores
    MAXT = max(1, int(Tgh.max()))

    # --- per-core edge index / slot arrays, call-ordered ---
    order = np.lexsort((slot, pig, half, grp, core))
    so = dict(core=core[order], grp=grp[order], half=half[order],
              pig=pig[order], slot=slot[order], idx16=idx16[order])
    run_start = np.zeros((n_cores, NGRP, 2), np.int64)
    flat_cnt = cnt3.reshape(-1)
    np.cumsum(flat_cnt[:-1], out=run_start.reshape(-1)[1:])

    # calls in (grp, half) order; record structure
    calls = []      # dicts: grp, h, T, tstart, idx_off
    tile_piggies = []  # per global tile: set of piggies present (union cores)
    tt = 0
    idxcols = 0
    for g in range(NGRP):
        for h in (0, 1):
            T = int(Tgh[g, h])
            if T == 0:
                continue
            calls.append(dict(grp=g, h=h, T=T, tstart=tt, idx_off=idxcols))
            for t in range(T):
                tile_piggies.append(set())
            tt += T
            idxcols += T * 8
    TT = tt
    IDXCOLS = idxcols
    call_of = {(c_["grp"], c_["h"]): i for i, c_ in enumerate(calls)}

    # trailing -1 pads are skipped by the gather ucode; per-core valid
    # counts are read at runtime (value_load) into num_idxs_reg
    idxflat = np.full((n_cores, TT * P), -1, np.int16)
    counts = np.ones((n_cores, max(1, len(calls))), np.int32)
    for c in range(n_cores):
        for ci, call in enumerate(calls):
            g, h, T = call["grp"], call["h"], call["T"]
            n = int(cnt3[c, g, h])
            s0 = int(run_start[c, g, h])
            o = call["tstart"] * P
            idxflat[c, o:o + n] = so["idx16"][s0:s0 + n].astype(np.int16)
            if n == 0:  # keep >=1 valid index (ucode/sim requirement)
                idxflat[c, o] = 0
            counts[c, ci] = max(n, 1)
            for t in range(T):
                a, b_ = t * P, min((t + 1) * P, n)
                if a >= n:
                    break
                pres = np.unique(so["pig"][s0 + a:s0 + b_])
                tile_piggies[call["tstart"] + t].update(int(p) for p in pres)

    for t in range(TT):
        if not tile_piggies[t]:
            tile_piggies[t].add(0)

    # matmul ids: one host-built one-hot panel per (tile, pig), grouped by grp
    mmid = np.full((TT, GROUP_NBLK), -1, np.int64)
    grp_mms = [[] for _ in range(NGRP)]  # (ci, t, gt, pig, m)
    m = 0
    for ci, call in enumerate(calls):
        for t in range(call["T"]):
            gt = call["tstart"] + t
            for pig in sorted(tile_piggies[gt]):
                mmid[gt, pig] = m
                grp_mms[call["grp"]].append((ci, t, gt, pig, m))
                m += 1
    NMM = m
    grp_moff = [min((e[4] for e in g_), default=0) for g_ in grp_mms]
    MAXM = max(len(g_) for g_ in grp_mms)

    # wrap-16 + replicate-to-128 index layout, call-local
    gidx = np.zeros((n_cores, P, IDXCOLS), np.int16)
    for call in calls:
        a = call["tstart"] * P
        T = call["T"]
        region = idxflat[:, a:a + T * P]
        arr = region.reshape(n_cores, T * 8, 16).transpose(0, 2, 1)
        gidx[:, :, call["idx_off"]:call["idx_off"] + T * 8] = (
            np.tile(arr, (1, 8, 1)))

    # host-built one-hot scatter panels (shared by both layers)
    bpans = []
    for c in range(n_cores):
        bp = np.zeros((P, NMM * P), BF16)
        for ci, call in enumerate(calls):
            g, h = call["grp"], call["h"]
            n = int(cnt3[c, g, h])
            if n == 0:
                continue
            s0 = int(run_start[c, g, h])
            p_ = np.arange(n)
            gt = call["tstart"] + p_ // P
            sp = p_ % P
            pig_e = so["pig"][s0:s0 + n]
            slot_e = so["slot"][s0:s0 + n]
            m_e = mmid[gt, pig_e]
            bp[sp, m_e * P + slot_e] = 1.0
        bpans.append(bp)

    # --- pooling helpers ---
    rows = np.arange(NP)
    rcore = rows // RPC
    rblk = (rows % RPC) // P
    rslot = rows % P
    pm = np.zeros((n_cores, P, BPC * G), BF16)
    rg = np.where(real, batch[np.clip(row2node, 0, N - 1)], -1)
    val = real
    pm[rcore[val], rslot[val], rblk[val] * G + rg[val]] = 1.0
    pmask = np.zeros((n_cores, P, G * BPC), BF16)
    for c in range(n_cores):
        for b in range(BPC):
            g = g_of_block[c * BPC + b]
            if g >= 0:
                pmask[c, :, g * BPC + b] = 1.0
    recip = (1.0 / np.maximum(cnt, 1.0)).astype(np.float32).reshape(G, 1)
    # last block position (over all cores) of each graph; empty graphs -> 0
    lastpos = np.zeros(G, np.int64)
    for bid in range(total_blocks_padded):
        g = g_of_block[bid]
        if g >= 0:
            lastpos[g] = max(lastpos[g], bid % BPC)
    graphs_at = [[] for _ in range(BPC)]
    for g in range(G):
        graphs_at[int(lastpos[g])].append(g)

    # --- per-core input maps ---
    in_maps = []
    for c in range(n_cores):
        r0, r1 = c * RPC, (c + 1) * RPC
        m = {
            "xt": np.ascontiguousarray(x_pad[r0:r1].T).astype(BF16),
            "w1": np.asarray(W1, np.float32).astype(BF16),
            "w2": np.asarray(W2, np.float32).astype(BF16),
            "wfc": np.asarray(Wfc, np.float32).astype(BF16),
            "b1r": np.asarray(b1, np.float32).reshape(1, FH).astype(BF16),
            "b2r": np.asarray(b2, np.float32).reshape(1, FH).astype(BF16),
            "bfcr": np.asarray(bfc, np.float32).reshape(1, FO).astype(BF16),
            "sqdeg": sqdeg_pad[r0:r1].reshape(1, RPC).astype(BF16),
            "dinv": np.ascontiguousarray(
                dinv_pad[r0:r1].reshape(BPC, P).T).astype(np.float32),
            "gidx": gidx[c],
            "counts": counts[c:c + 1],
            "bpan": bpans[c],
            "pm": pm[c],
            "pmask": pmask[c],
            "recip": recip,
        }
        in_maps.append(m)

    plan = dict(
        G=G, F=F, FH=FH, FO=FO, BPC=BPC, RPC=RPC, NP=NP, HALF=HALF,
        HROWS=HROWS, NGRP=NGRP, TT=TT, IDXCOLS=IDXCOLS, MAXT=MAXT,
        calls=calls, call_of=call_of,
        grp_mms=grp_mms, grp_moff=grp_moff, NMM=NMM, MAXM=MAXM,
        graphs_at=graphs_at,
        n_cores=n_cores,
        has_b1=bool(np.any(np.asarray(b1))),
        has_b2=bool(np.any(np.asarray(b2))),
        has_bfc=bool(np.any(np.asarray(bfc))),
    )
    return plan, in_maps


# --------------------------------------------------------------------------
# Bass program builder (identical on all cores).
# --------------------------------------------------------------------------

def build(plan, debug=False):
    dt = mybir.dt
    G, F, FH, FO = plan["G"], plan["F"], plan["FH"], plan["FO"]
    BPC, RPC, NP = plan["BPC"], plan["RPC"], plan["NP"]
    HROWS, NGRP = plan["HROWS"], plan["NGRP"]
    TT, IDXCOLS, MAXT = plan["TT"], plan["IDXCOLS"], plan["MAXT"]
    calls, call_of = plan["calls"], plan["call_of"]
    grp_mms, grp_moff = plan["grp_mms"], plan["grp_moff"]
    NMM, MAXM = plan["NMM"], plan["MAXM"]
    graphs_at = plan["graphs_at"]
    n_cores = plan["n_cores"]
    KC = F // P          # k-chunks for the transforms (2)
    FCK = (3 * FH) // P  # k-chunks for the FC (6)
    HB = BPC // 2        # blocks per table half

    nc = bacc.Bacc("TRN2", target_bir_lowering=False, debug=debug,
                   num_devices=n_cores)

    def din(name, shape, dtype):
        return nc.dram_tensor(name, shape, dtype, kind="ExternalInput").ap()

    xt_d = din("xt", [F, RPC], dt.bfloat16)
    w1_d = din("w1", [F, FH], dt.bfloat16)
    w2_d = din("w2", [FH, FH], dt.bfloat16)
    wfc_d = din("wfc", [3 * FH, FO], dt.bfloat16)
    b1r_d = din("b1r", [1, FH], dt.bfloat16)
    b2r_d = din("b2r", [1, FH], dt.bfloat16)
    bfcr_d = din("bfcr", [1, FO], dt.bfloat16)
    sqdeg_d = din("sqdeg", [1, RPC], dt.bfloat16)
    dinv_d = din("dinv", [P, BPC], dt.float32)
    gidx_d = din("gidx", [P, IDXCOLS], dt.int16)
    counts_d = din("counts", [1, max(1, len(calls))], dt.int32)
    bpan_d = din("bpan", [P, NMM * P], dt.bfloat16)
    pm_d = din("pm", [P, BPC * G], dt.bfloat16)
    pmask_d = din("pmask", [P, G * BPC], dt.bfloat16)
    recip_d = din("recip", [G, 1], dt.float32)
    out_d = nc.dram_tensor("out", [G, FO], dt.float32,
                           kind="ExternalOutput").ap()

    rg = [list(range(n_cores))]

    from contextlib import ExitStack
    with tile.TileContext(nc) as tc, ExitStack() as ctx:
        const = ctx.enter_context(tc.tile_pool(name="const", bufs=1))
        dram = ctx.enter_context(tc.tile_pool(name="dram", bufs=1, space="DRAM"))
        tfpsum = ctx.enter_context(tc.tile_pool(name="tfpsum", bufs=2, space="PSUM"))
        aggpsum = ctx.enter_context(tc.tile_pool(name="aggpsum", bufs=3, space="PSUM"))
        tpsum = ctx.enter_context(tc.tile_pool(name="tpsum", bufs=1, space="PSUM"))
        spsum = ctx.enter_context(tc.tile_pool(name="spsum", bufs=1, space="PSUM"))
        fcpsum = ctx.enter_context(tc.tile_pool(name="fcpsum", bufs=1, space="PSUM"))
        msgp = ctx.enter_context(tc.tile_pool(name="msgp", bufs=5))
        bpp = ctx.enter_context(tc.tile_pool(name="bpp", bufs=2))
        btp = ctx.enter_context(tc.tile_pool(name="btp", bufs=4))
        hp = ctx.enter_context(tc.tile_pool(name="hp", bufs=3))
        htp = ctx.enter_context(tc.tile_pool(name="htp", bufs=4))
        tailp = ctx.enter_context(tc.tile_pool(name="tailp", bufs=1))

        # ---------------- constants into SBUF ----------------
        def cload(tag, dram_ap, shape, dtype):
            t = const.tile(shape, dtype, tag=tag)
            nc.sync.dma_start(out=t[:], in_=dram_ap)
            return t

        # trigger the first (dummy) collective ASAP: the runtime's one-time
        # collective init (~60us) runs serially before the first real
        # AllGather otherwise
        warm_in = dram.tile([8, 16], dt.bfloat16, tag="warmin")
        warm_out = dram.tile([64, 16], dt.bfloat16, tag="warmout",
                             addr_space="Shared")
        wz = const.tile([8, 16], dt.bfloat16, tag="wz")
        nc.gpsimd.memset(wz[:], 0.0)
        nc.sync.dma_start(out=warm_in[:], in_=wz[:])
        nc.gpsimd.collective_compute(
            "AllGather", mybir.AluOpType.bypass,
            ins=[warm_in[:].opt()], outs=[warm_out[:].opt()],
            replica_groups=rg)

        # zero-fill the msg ring once (skipped -1 pad slots must read
        # finite); allocated here so the ring sits at low SBUF offsets
        for _ in range(5):
            mb0 = msgp.tile([P, MAXT * FH], dt.bfloat16, tag="msg")
            nc.gpsimd.memset(mb0[:], 0.0)

        gidx_sb = const.tile([P, IDXCOLS], dt.int16, tag="gidx")
        counts_sb = const.tile([1, max(1, len(calls))], dt.int32,
                               tag="counts")
        xt_sb = const.tile([P, KC * RPC], dt.bfloat16, tag="xt")
        for c in range(KC):
            nc.sync.dma_start(out=xt_sb[:, c * RPC:(c + 1) * RPC],
                              in_=xt_d[c * P:(c + 1) * P, :])
        w_sb = []
        for tag, d in (("w1", w1_d), ("w2", w2_d)):
            t = const.tile([P, KC * FH], dt.bfloat16, tag=tag)
            for c in range(KC):
                nc.sync.dma_start(out=t[:, c * FH:(c + 1) * FH],
                                  in_=d[c * P:(c + 1) * P, :])
            w_sb.append(t)
        wfc_sb = const.tile([P, FCK * FO], dt.bfloat16, tag="wfc")
        for c in range(FCK):
            nc.sync.dma_start(out=wfc_sb[:, c * FO:(c + 1) * FO],
                              in_=wfc_d[c * P:(c + 1) * P, :])
        dinv_sb = cload("dinv", dinv_d, [P, BPC], dt.float32)

        iota_sb = const.tile([P, P], dt.float32, tag="iota")
        nc.gpsimd.iota(out=iota_sb[:], pattern=[[1, P]], base=0,
                       channel_multiplier=0,
                       allow_small_or_imprecise_dtypes=True)
        iotac_sb = const.tile([P, 1], dt.float32, tag="iotac")
        nc.gpsimd.iota(out=iotac_sb[:], pattern=[[0, 1]], base=0,
                       channel_multiplier=1,
                       allow_small_or_imprecise_dtypes=True)
        ident_sb = const.tile([P, P], dt.bfloat16, tag="ident")
        nc.vector.tensor_tensor(out=ident_sb[:],
                                in0=iotac_sb[:].to_broadcast([P, P]),
                                in1=iota_sb[:],
                                op=mybir.AluOpType.is_equal)
        ones_sb = const.tile([1, G], dt.bfloat16, tag="ones")
        nc.gpsimd.memset(ones_sb[:], 1.0)
        blockmax = const.tile([P, KC * BPC], dt.bfloat16, tag="bmax")
        nc.gpsimd.memset(blockmax[:], 0.0)
        # persistent per-layer local tables (bf16, dinv-scaled)
        tbl = [const.tile([P, BPC * FH], dt.bfloat16, name=f"tbl{l}",
                          tag=f"tbl{l}")
               for l in range(2)]

        # DRAM bounce buffers for collectives (per layer, per half)
        ag_in = [[dram.tile([HROWS, FH], dt.bfloat16, name=f"agin{l}{h}",
                            tag=f"agin{l}{h}")
                  for h in (0, 1)] for l in range(2)]
        ag_out = [[dram.tile([HROWS * n_cores, FH], dt.bfloat16,
                             name=f"agout{l}{h}", tag=f"agout{l}{h}")
                   for h in (0, 1)] for l in range(2)]
        ars_in = [dram.tile([G, FH], dt.bfloat16, name=f"arsin{k}",
                            tag=f"arsin{k}") for k in (0, 1)]
        ars_out = [dram.tile([G, FH], dt.bfloat16, name=f"arsout{k}",
                             tag=f"arsout{k}", addr_space="Shared")
                   for k in (0, 1)]
        arm_in = dram.tile([P, KC * G], dt.bfloat16, tag="armin")
        arm_out = dram.tile([P, KC * G], dt.bfloat16, tag="armout",
                            addr_space="Shared")

        Copy = mybir.ActivationFunctionType.Copy
        Relu = mybir.ActivationFunctionType.Relu

        cnt_reg = nc.gpsimd.alloc_register("cntreg")

        def allgather(l, h):
            nc.gpsimd.collective_compute(
                "AllGather", mybir.AluOpType.bypass,
                ins=[ag_in[l][h][:].opt()], outs=[ag_out[l][h][:].opt()],
                replica_groups=rg)

        def push_block(l, b):
            h = 0 if b < HB else 1
            r0 = (b - h * HB) * P
            nc.scalar.dma_start(out=ag_in[l][h][r0:r0 + P, :],
                                in_=tbl[l][:, b * FH:(b + 1) * FH])

        # ---------------- layer-1 transform ----------------
        for b in range(BPC):
            ps = tfpsum.tile([P, FH], dt.float32, tag="tfps")
            for c in range(KC):
                nc.tensor.matmul(
                    out=ps[:],
                    lhsT=xt_sb[:, c * RPC + b * P:c * RPC + (b + 1) * P],
                    rhs=w_sb[0][:, c * FH:(c + 1) * FH],
                    start=(c == 0), stop=(c == KC - 1))
            nc.scalar.activation(out=tbl[0][:, b * FH:(b + 1) * FH],
                                 in_=ps[:], func=Copy,
                                 scale=dinv_sb[:, b:b + 1])
            push_block(0, b)
            if b == HB - 1:
                allgather(0, 0)
        allgather(0, 1)

        # deferred const loads (needed from the agg phase on, not by the
        # transforms -- keeps the early sync-DMA queue clear so the first
        # AllGather can fire as soon as the transforms finish)
        nc.sync.dma_start(out=gidx_sb[:], in_=gidx_d)
        nc.sync.dma_start(out=counts_sb[:], in_=counts_d)
        b1r_sb = cload("b1r", b1r_d, [1, FH], dt.bfloat16)
        b2r_sb = cload("b2r", b2r_d, [1, FH], dt.bfloat16)
        bfcr_sb = cload("bfcr", bfcr_d, [1, FO], dt.bfloat16)
        sqdeg_sb = cload("sqdeg", sqdeg_d, [1, RPC], dt.bfloat16)
        pm_sb = cload("pm", pm_d, [P, BPC * G], dt.bfloat16)
        pmask_sb = cload("pmask", pmask_d, [P, G * BPC], dt.bfloat16)
        recip_sb = cload("recip", recip_d, [G, 1], dt.float32)

        # ---------------- aggregation over edges ----------------
        AWIN = 3  # h=0 calls issued ahead of h=1 calls (hides AG of half 1)

        def agg_layer(l, bias_row, has_bias, produce_block):
            # interleave calls: [g0A g1A g2A g0B g1B ...] with groups'
            # matmul work following once both halves of a group are in.
            mbufs = {}   # call index -> sbuf tile

            def do_call(ci):
                call = calls[ci]
                T = call["T"]
                nc.gpsimd.reg_load(cnt_reg, counts_sb[0:1, ci:ci + 1])
                mb = msgp.tile([P, MAXT * FH], dt.bfloat16, tag="msg")
                out_ap = mb[:, :T * FH].rearrange("p (t e) -> p t e", e=FH)
                nc.gpsimd.dma_gather(
                    out_ap=out_ap,
                    in_ap=ag_out[l][call["h"]][:],
                    idxs_ap=gidx_sb[:, call["idx_off"]:
                                    call["idx_off"] + T * 8],
                    num_idxs=T * P,
                    num_idxs_reg=cnt_reg,
                    elem_size=FH,
                    single_packet=False)
                mbufs[ci] = mb

            order = []   # call issue order (A-window interleave)
            apend = [ci for ci in range(len(calls)) if calls[ci]["h"] == 0]
            bpend = [ci for ci in range(len(calls)) if calls[ci]["h"] == 1]
            ai = bi = 0
            while ai < len(apend) or bi < len(bpend):
                win = AWIN + 1 if bi == 0 else AWIN
                if ai < len(apend) and (ai - bi < win or bi >= len(bpend)):
                    order.append(apend[ai]); ai += 1
                else:
                    order.append(bpend[bi]); bi += 1

            done_upto = 0  # groups fully processed

            def group_ready(g):
                for h in (0, 1):
                    ci = call_of.get((g, h))
                    if ci is not None and ci not in mbufs:
                        return False
                return True

            def process_group(g):
                nmm = len(grp_mms[g])
                moff = grp_moff[g]
                bsl = None
                if nmm:
                    bsl = bpp.tile([P, MAXM * P], dt.bfloat16, tag="bsl")
                    nc.sync.dma_start(
                        out=bsl[:, :nmm * P],
                        in_=bpan_d[:, moff * P:(moff + nmm) * P])
                for pig in range(GROUP_NBLK):
                    b = g * GROUP_NBLK + pig
                    ps = aggpsum.tile([P, FH], dt.float32, tag="aggps")
                    mms = [e for e in grp_mms[g] if e[3] == pig]
                    # self loop: identity matmul against local table block
                    nc.tensor.matmul(out=ps[:], lhsT=ident_sb[:],
                                     rhs=tbl[l][:, b * FH:(b + 1) * FH],
                                     start=True,
                                     stop=(not mms) and not has_bias)
                    for k, (ci, t, gt, _pig, m) in enumerate(mms):
                        mo = m - moff
                        nc.tensor.matmul(
                            out=ps[:], lhsT=bsl[:, mo * P:(mo + 1) * P],
                            rhs=mbufs[ci][:, t * FH:(t + 1) * FH],
                            start=False,
                            stop=(k == len(mms) - 1) and not has_bias)
                    if has_bias:
                        nc.tensor.matmul(
                            out=ps[:],
                            lhsT=sqdeg_sb[:, b * P:(b + 1) * P],
                            rhs=bias_row[:],
                            start=False, stop=True)
                    produce_block(b, ps)

            for ci in order:
                do_call(ci)
                # process any groups that are now complete, in order
                while done_upto < NGRP and group_ready(done_upto):
                    process_group(done_upto)
                    g = done_upto
                    done_upto += 1
                    # release msg buffers of this group
                    for h in (0, 1):
                        cix = call_of.get((g, h))
                        if cix in mbufs:
                            del mbufs[cix]
                    if l == 0 and g == min(NGRP - 1, HB // GROUP_NBLK + 2):
                        allgather(1, 0)
            while done_upto < NGRP:
                process_group(done_upto)
                done_upto += 1
            if l == 0:
                allgather(1, 1)

        # layer-1 block epilogue: relu, transform to layer-2 table
        def produce1(b, ps):
            h1 = hp.tile([P, FH], dt.bfloat16, tag="h1")
            nc.scalar.activation(out=h1[:], in_=ps[:], func=Relu,
                                 scale=dinv_sb[:, b:b + 1])
            h1t = []
            for c in range(KC):
                tp = tpsum.tile([P, P], dt.bfloat16, tag="tp")
                nc.tensor.transpose(out=tp[:],
                                    in_=h1[:, c * P:(c + 1) * P],
                                    identity=ident_sb[:])
                ht = htp.tile([P, P], dt.bfloat16, tag="ht")
                nc.vector.tensor_copy(out=ht[:], in_=tp[:])
                h1t.append(ht)
            ps2 = tfpsum.tile([P, FH], dt.float32, tag="tfps")
            for c in range(KC):
                nc.tensor.matmul(out=ps2[:], lhsT=h1t[c][:],
                                 rhs=w_sb[1][:, c * FH:(c + 1) * FH],
                                 start=(c == 0), stop=(c == KC - 1))
            nc.scalar.activation(out=tbl[1][:, b * FH:(b + 1) * FH],
                                 in_=ps2[:], func=Copy,
                                 scale=dinv_sb[:, b:b + 1])
            push_block(1, b)

        agg_layer(0, b1r_sb, plan["has_b1"], produce1)

        # layer-2 block epilogue: relu, pooling contributions
        sums_ps = spsum.tile([G, FH], dt.float32, tag="sums")
        mxT_loc = const.tile([P, KC * G], dt.bfloat16, tag="mxT_loc")

        SUMS_SPLIT = BPC - 6 if BPC > 8 else -1  # no split on tiny configs

        def produce2(b, ps):
            h2 = hp.tile([P, FH], dt.bfloat16, tag="h2")
            nc.scalar.activation(out=h2[:], in_=ps[:], func=Relu,
                                 scale=dinv_sb[:, b:b + 1])
            nc.tensor.matmul(out=sums_ps[:],
                             lhsT=pm_sb[:, b * G:(b + 1) * G],
                             rhs=h2[:],
                             start=(b == 0 or b == SUMS_SPLIT),
                             stop=(b == SUMS_SPLIT - 1 or b == BPC - 1))
            if b == SUMS_SPLIT - 1:
                ssA = tailp.tile([G, FH], dt.bfloat16, tag="ssA")
                nc.vector.tensor_copy(out=ssA[:], in_=sums_ps[:])
                nc.sync.dma_start(out=ars_in[0][:], in_=ssA[:])
                nc.gpsimd.collective_compute(
                    "AllReduce", mybir.AluOpType.add,
                    ins=[ars_in[0][:].opt()], outs=[ars_out[0][:].opt()],
                    replica_groups=rg)
            for c in range(KC):
                tp = tpsum.tile([P, P], dt.bfloat16, tag="tp")
                nc.tensor.transpose(out=tp[:],
                                    in_=h2[:, c * P:(c + 1) * P],
                                    identity=ident_sb[:])
                nc.vector.tensor_reduce(
                    out=blockmax[:, c * BPC + b:c * BPC + b + 1],
                    in_=tp[:], axis=mybir.AxisListType.X,
                    op=mybir.AluOpType.max)
            # per-graph local max for graphs whose last block is b
            for g in graphs_at[b]:
                mtmp = btp.tile([P, BPC], dt.bfloat16, tag="mtmp")
                for c in range(KC):
                    nc.vector.tensor_tensor(
                        out=mtmp[:],
                        in0=blockmax[:, c * BPC:(c + 1) * BPC],
                        in1=pmask_sb[:, g * BPC:(g + 1) * BPC],
                        op=mybir.AluOpType.mult)
                    nc.vector.tensor_reduce(
                        out=mxT_loc[:, c * G + g:c * G + g + 1],
                        in_=mtmp[:],
                        axis=mybir.AxisListType.X, op=mybir.AluOpType.max)

        agg_layer(1, b2r_sb, plan["has_b2"], produce2)

        # ---------------- pooling tail ----------------
        sums_sb = tailp.tile([G, FH], dt.bfloat16, tag="sums_sb")
        nc.vector.tensor_copy(out=sums_sb[:], in_=sums_ps[:])
        nc.sync.dma_start(out=ars_in[1][:], in_=sums_sb[:])
        nc.gpsimd.collective_compute(
            "AllReduce", mybir.AluOpType.add,
            ins=[ars_in[1][:].opt()], outs=[ars_out[1][:].opt()],
            replica_groups=rg)
        nc.sync.dma_start(out=arm_in[:], in_=mxT_loc[:])
        nc.gpsimd.collective_compute(
            "AllReduce", mybir.AluOpType.max,
            ins=[arm_in[:].opt()], outs=[arm_out[:].opt()],
            replica_groups=rg)

        gsA = tailp.tile([G, FH], dt.bfloat16, tag="gsA")
        if SUMS_SPLIT > 0:
            nc.sync.dma_start(out=gsA[:], in_=ars_out[0][:])
        else:
            nc.gpsimd.memset(gsA[:], 0.0)
        gsB = tailp.tile([G, FH], dt.bfloat16, tag="gsB")
        nc.sync.dma_start(out=gsB[:], in_=ars_out[1][:])
        gsums = tailp.tile([G, FH], dt.bfloat16, tag="gsums")
        nc.vector.tensor_tensor(out=gsums[:], in0=gsA[:], in1=gsB[:],
                                op=mybir.AluOpType.add)
        mxT = tailp.tile([P, KC * G], dt.bfloat16, tag="mxT")
        nc.sync.dma_start(out=mxT[:], in_=arm_out[:])

        # mean / sums in bf16, transposed to feature-major for the FC
        mean_sb = tailp.tile([G, FH], dt.bfloat16, tag="mean")
        nc.vector.tensor_scalar(out=mean_sb[:], in0=gsums[:],
                                scalar1=recip_sb[:], scalar2=None,
                                op0=mybir.AluOpType.mult)
        sums_bf = tailp.tile([G, FH], dt.bfloat16, tag="sumsbf")
        nc.vector.tensor_copy(out=sums_bf[:], in_=gsums[:])
        meanT = tailp.tile([P, KC * G], dt.bfloat16, tag="meanT")
        sumsT = tailp.tile([P, KC * G], dt.bfloat16, tag="sumsT")
        for src, dst_t in ((mean_sb, meanT), (sums_bf, sumsT)):
            for c in range(KC):
                tp = tpsum.tile([P, P], dt.bfloat16, tag="tp")
                nc.tensor.transpose(out=tp[:, :G],
                                    in_=src[:, c * P:(c + 1) * P],
                                    identity=ident_sb[:G, :G])
                nc.vector.tensor_copy(out=dst_t[:, c * G:(c + 1) * G],
                                      in_=tp[:, :G])

        # final FC: out = [mean | max | sums] @ Wfc + bfc
        fc_ps = fcpsum.tile([G, FO], dt.float32, tag="fc")
        gT = [meanT, mxT, sumsT]
        k = 0
        for part in range(3):
            for c in range(KC):
                nc.tensor.matmul(
                    out=fc_ps[:], lhsT=gT[part][:, c * G:(c + 1) * G],
                    rhs=wfc_sb[:, k * FO:(k + 1) * FO],
                    start=(k == 0),
                    stop=(k == FCK - 1) and not plan["has_bfc"])
                k += 1
        if plan["has_bfc"]:
            nc.tensor.matmul(out=fc_ps[:], lhsT=ones_sb[:], rhs=bfcr_sb[:],
                             start=False, stop=True)
        out_sb = tailp.tile([G, FO], dt.float32, tag="out_sb")
        nc.vector.tensor_copy(out=out_sb[:], in_=fc_ps[:])
        nc.sync.dma_start(out=out_d[:], in_=out_sb[:])

    nc.compile()
    return nc


# --------------------------------------------------------------------------
# Entry point for the grading harness.
# --------------------------------------------------------------------------

def kernel(x, edge_index, batch, n_graphs, W1, b1, W2, b2, Wfc, bfc,
           **_unused):
    plan, in_maps = preprocess(x, edge_index, batch, n_graphs,
                               W1, b1, W2, b2, Wfc, bfc)
    nc = build(plan)
    res = run_bass_kernel_spmd(nc, in_maps, core_ids=list(range(NCORES)))
    out = np.asarray(res.results[0]["out"], np.float32)
    return out


# revision 44
# speedup vs baseline: 1.1632x; 1.0025x over previous
"""Trainium2 Bass kernel for a 2-layer GCN (EnhancedGNN) with triple global
pooling and a final FC, run SPMD across 8 NeuronCores.

Strategy:
  - Nodes are re-ordered so every 128-row block belongs to exactly one graph
    ("pure blocks"), padded per-graph to multiples of 128. Blocks are sharded
    contiguously across the 8 cores (dst / data parallel).
  - Per layer: each core transforms its node shard (x @ W, scaled by
    dinv = deg^-1/2) into a bf16 "table" shard kept in SBUF AND pushed to
    DRAM; two AllGathers (one per half of the shard) replicate the table to
    every core so gathers on the first half can start while the second half
    is still in flight.
  - Aggregation: per group of 2 dst blocks and per table half, one
    dma_gather fetches the 512B message rows for all edges of the group
    (edges laid contiguously, slot-coded with a 256*block offset); messages
    are scattered into per-block PSUM accumulators with one-hot matmuls
    (B built on DVE via is_equal against offset iotas). Self-loop terms use
    an identity matmul against the SBUF-resident local table block (no
    gather, and it initializes the PSUM).
  - Pooling: per-graph sums via a one-hot matmul, per-graph max via
    per-block feature-major reduce_max + data-driven graph masks evaluated
    incrementally (as soon as a graph's last block is aggregated), then
    AllReduce(add/max) and a tiny FC run redundantly on every core.

The kernel program is identical on all 8 cores (SPMD); all per-core
differences live in the input data. Structure constants (tile counts etc.)
are maxima over cores so the program is uniform.
"""

import numpy as np
import ml_dtypes

import concourse.bass as bass
import concourse.tile as tile
from concourse import bacc, mybir
from concourse.bass_utils import run_bass_kernel_spmd

P = 128
NCORES = 8
GROUP_NBLK = 2  # dst blocks per gather group

BF16 = ml_dtypes.bfloat16
PAD_SLOT = 1000.0  # slot code that never matches any iota offset


def _cdiv(a, b):
    return -(-a // b)


# --------------------------------------------------------------------------
# Host-side preprocessing: sharding, edge grouping, auxiliary tensors.
# --------------------------------------------------------------------------

def preprocess(x, edge_index, batch, n_graphs, W1, b1, W2, b2, Wfc, bfc,
               n_cores=NCORES):
    x = np.asarray(x, np.float32)
    ei = np.asarray(edge_index, np.int64)
    batch = np.asarray(batch, np.int64)
    G = int(n_graphs)
    N = x.shape[0]
    F = x.shape[1]
    FH = W1.shape[1]
    FO = Wfc.shape[1]
    assert F == FH, "kernel assumes F_IN == F_HID"

    # degrees (dst side, + self loop), as in the reference
    deg = np.bincount(ei[1], minlength=N).astype(np.float32) + 1.0
    dinv = 1.0 / np.sqrt(deg)
    sqdeg = np.sqrt(deg)

    # --- graph-padded node ordering (pure blocks) ---
    cnt = np.bincount(batch, minlength=G).astype(np.int64)  # nodes per graph
    blocks_g = _cdiv(cnt, P)  # 0 for empty graphs
    total_blocks = int(blocks_g.sum())
    # pad so BPC is even (needed for the half-split of each core's shard)
    total_blocks_padded = _cdiv(total_blocks, 2 * n_cores) * 2 * n_cores
    BPC = total_blocks_padded // n_cores
    RPC = BPC * P
    NP = total_blocks_padded * P
    HROWS = RPC // 2            # local rows per table half
    HALF = NP // 2              # rows per table half (all cores)
    assert HALF <= 32768, f"table half {HALF} exceeds int16 index range"
    NGRP = BPC // GROUP_NBLK
    assert BPC % GROUP_NBLK == 0

    blk_start = np.concatenate([[0], np.cumsum(blocks_g)])  # per graph
    row_start = blk_start * P
    first_node = np.concatenate([[0], np.cumsum(cnt)])[:-1]
    new_pos = row_start[batch] + (np.arange(N) - first_node[batch])
    row2node = np.full(NP, -1, np.int64)
    row2node[new_pos] = np.arange(N)
    real = row2node >= 0

    # per padded row data
    x_pad = np.zeros((NP, F), np.float32)
    x_pad[real] = x[row2node[real]]
    dinv_pad = np.ones(NP, np.float32)
    dinv_pad[real] = dinv[row2node[real]]
    sqdeg_pad = np.zeros(NP, np.float32)
    sqdeg_pad[real] = sqdeg[row2node[real]]
    g_of_block = np.full(total_blocks_padded, -1, np.int64)
    for g in range(G):
        g_of_block[blk_start[g]:blk_start[g + 1]] = g

    # --- edges (real edges only; +I loops handled by identity matmuls) ---
    es = new_pos[ei[0]]
    ed = new_pos[ei[1]]
    core = ed // RPC
    pos = (ed % RPC) // P          # block position within core
    grp = pos // GROUP_NBLK
    pig = pos % GROUP_NBLK         # position in group
    slot = ed % P
    lr = es % RPC                  # src local row on its owner core
    half = lr // HROWS
    idx16 = (es // RPC) * HROWS + (lr - half * HROWS)

    # counts per (core, group, half)
    cnt3 = np.zeros((n_cores, NGRP, 2), np.int64)
    np.add.at(cnt3, (core, grp, half), 1)
    Tgh = _cdiv(cnt3.max(axis=0), P)  # [NGRP, 2] tiles, uniform across cores
    MAXT = max(1, int(Tgh.max()))

    # --- per-core edge index / slot arrays, call-ordered ---
    order = np.lexsort((slot, pig, half, grp, core))
    so = dict(core=core[order], grp=grp[order], half=half[order],
              pig=pig[order], slot=slot[order], idx16=idx16[order])
    run_start = np.zeros((n_cores, NGRP, 2), np.int64)
    flat_cnt = cnt3.reshape(-1)
    np.cumsum(flat_cnt[:-1], out=run_start.reshape(-1)[1:])

    # calls in (grp, half) order; record structure
    calls = []      # dicts: grp, h, T, tstart, idx_off
    tile_piggies = []  # per global tile: set of piggies present (union cores)
    tt = 0
    idxcols = 0
    for g in range(NGRP):
        for h in (0, 1):
            T = int(Tgh[g, h])
            if T == 0:
                continue
            calls.append(dict(grp=g, h=h, T=T, tstart=tt, idx_off=idxcols))
            for t in range(T):
                tile_piggies.append(set())
            tt += T
            idxcols += T * 8
    TT = tt
    IDXCOLS = idxcols
    call_of = {(c_["grp"], c_["h"]): i for i, c_ in enumerate(calls)}

    # trailing -1 pads are skipped by the gather ucode; per-core valid
    # counts are read at runtime (value_load) into num_idxs_reg
    idxflat = np.full((n_cores, TT * P), -1, np.int16)
    counts = np.ones((n_cores, max(1, len(calls))), np.int32)
    for c in range(n_cores):
        for ci, call in enumerate(calls):
            g, h, T = call["grp"], call["h"], call["T"]
            n = int(cnt3[c, g, h])
            s0 = int(run_start[c, g, h])
            o = call["tstart"] * P
            idxflat[c, o:o + n] = so["idx16"][s0:s0 + n].astype(np.int16)
            if n == 0:  # keep >=1 valid index (ucode/sim requirement)
                idxflat[c, o] = 0
            counts[c, ci] = max(n, 1)
            for t in range(T):
                a, b_ = t * P, min((t + 1) * P, n)
                if a >= n:
                    break
                pres = np.unique(so["pig"][s0 + a:s0 + b_])
                tile_piggies[call["tstart"] + t].update(int(p) for p in pres)

    for t in range(TT):
        if not tile_piggies[t]:
            tile_piggies[t].add(0)

    # matmul ids: one host-built one-hot panel per (tile, pig), grouped by grp
    mmid = np.full((TT, GROUP_NBLK), -1, np.int64)
    grp_mms = [[] for _ in range(NGRP)]  # (ci, t, gt, pig, m)
    m = 0
    for ci, call in enumerate(calls):
        for t in range(call["T"]):
            gt = call["tstart"] + t
            for pig in sorted(tile_piggies[gt]):
                mmid[gt, pig] = m
                grp_mms[call["grp"]].append((ci, t, gt, pig, m))
                m += 1
    NMM = m
    grp_moff = [min((e[4] for e in g_), default=0) for g_ in grp_mms]
    MAXM = max(len(g_) for g_ in grp_mms)

    # wrap-16 + replicate-to-128 index layout, call-local
    gidx = np.zeros((n_cores, P, IDXCOLS), np.int16)
    for call in calls:
        a = call["tstart"] * P
        T = call["T"]
        region = idxflat[:, a:a + T * P]
        arr = region.reshape(n_cores, T * 8, 16).transpose(0, 2, 1)
        gidx[:, :, call["idx_off"]:call["idx_off"] + T * 8] = (
            np.tile(arr, (1, 8, 1)))

    # host-built one-hot scatter panels (shared by both layers)
    bpans = []
    for c in range(n_cores):
        bp = np.zeros((P, NMM * P), BF16)
        for ci, call in enumerate(calls):
            g, h = call["grp"], call["h"]
            n = int(cnt3[c, g, h])
            if n == 0:
                continue
            s0 = int(run_start[c, g, h])
            p_ = np.arange(n)
            gt = call["tstart"] + p_ // P
            sp = p_ % P
            pig_e = so["pig"][s0:s0 + n]
            slot_e = so["slot"][s0:s0 + n]
            m_e = mmid[gt, pig_e]
            bp[sp, m_e * P + slot_e] = 1.0
        bpans.append(bp)

    # --- pooling helpers ---
    rows = np.arange(NP)
    rcore = rows // RPC
    rblk = (rows % RPC) // P
    rslot = rows % P
    pm = np.zeros((n_cores, P, BPC * G), BF16)
    rg = np.where(real, batch[np.clip(row2node, 0, N - 1)], -1)
    val = real
    pm[rcore[val], rslot[val], rblk[val] * G + rg[val]] = 1.0
    pmask = np.zeros((n_cores, P, G * BPC), BF16)
    for c in range(n_cores):
        for b in range(BPC):
            g = g_of_block[c * BPC + b]
            if g >= 0:
                pmask[c, :, g * BPC + b] = 1.0
    recip = (1.0 / np.maximum(cnt, 1.0)).astype(np.float32).reshape(G, 1)
    # last block position (over all cores) of each graph; empty graphs -> 0
    lastpos = np.zeros(G, np.int64)
    for bid in range(total_blocks_padded):
        g = g_of_block[bid]
        if g >= 0:
            lastpos[g] = max(lastpos[g], bid % BPC)
    graphs_at = [[] for _ in range(BPC)]
    for g in range(G):
        graphs_at[int(lastpos[g])].append(g)

    # --- per-core input maps ---
    in_maps = []
    for c in range(n_cores):
        r0, r1 = c * RPC, (c + 1) * RPC
        m = {
            "xt": np.ascontiguousarray(x_pad[r0:r1].T).astype(BF16),
            "w1": np.asarray(W1, np.float32).astype(BF16),
            "w2": np.asarray(W2, np.float32).astype(BF16),
            "wfc": np.asarray(Wfc, np.float32).astype(BF16),
            "b1r": np.asarray(b1, np.float32).reshape(1, FH).astype(BF16),
            "b2r": np.asarray(b2, np.float32).reshape(1, FH).astype(BF16),
            "bfcr": np.asarray(bfc, np.float32).reshape(1, FO).astype(BF16),
            "sqdeg": sqdeg_pad[r0:r1].reshape(1, RPC).astype(BF16),
            "dinv": np.ascontiguousarray(
                dinv_pad[r0:r1].reshape(BPC, P).T).astype(np.float32),
            "gidx": gidx[c],
            "counts": counts[c:c + 1],
            "bpan": bpans[c],
            "pm": pm[c],
            "pmask": pmask[c],
            "recip": recip,
        }
        in_maps.append(m)

    plan = dict(
        G=G, F=F, FH=FH, FO=FO, BPC=BPC, RPC=RPC, NP=NP, HALF=HALF,
        HROWS=HROWS, NGRP=NGRP, TT=TT, IDXCOLS=IDXCOLS, MAXT=MAXT,
        calls=calls, call_of=call_of,
        grp_mms=grp_mms, grp_moff=grp_moff, NMM=NMM, MAXM=MAXM,
        graphs_at=graphs_at,
        n_cores=n_cores,
        has_b1=bool(np.any(np.asarray(b1))),
        has_b2=bool(np.any(np.asarray(b2))),
        has_bfc=bool(np.any(np.asarray(bfc))),
    )
    return plan, in_maps


# --------------------------------------------------------------------------
# Bass program builder (identical on all cores).
# --------------------------------------------------------------------------

def build(plan, debug=False):
    dt = mybir.dt
    G, F, FH, FO = plan["G"], plan["F"], plan["FH"], plan["FO"]
    BPC, RPC, NP = plan["BPC"], plan["RPC"], plan["NP"]
    HROWS, NGRP = plan["HROWS"], plan["NGRP"]
    TT, IDXCOLS, MAXT = plan["TT"], plan["IDXCOLS"], plan["MAXT"]
    calls, call_of = plan["calls"], plan["call_of"]
    grp_mms, grp_moff = plan["grp_mms"], plan["grp_moff"]
    NMM, MAXM = plan["NMM"], plan["MAXM"]
    graphs_at = plan["graphs_at"]
    n_cores = plan["n_cores"]
    KC = F // P          # k-chunks for the transforms (2)
    FCK = (3 * FH) // P  # k-chunks for the FC (6)
    HB = BPC // 2        # blocks per table half

    nc = bacc.Bacc("TRN2", target_bir_lowering=False, debug=debug,
                   num_devices=n_cores)

    def din(name, shape, dtype):
        return nc.dram_tensor(name, shape, dtype, kind="ExternalInput").ap()

    xt_d = din("xt", [F, RPC], dt.bfloat16)
    w1_d = din("w1", [F, FH], dt.bfloat16)
    w2_d = din("w2", [FH, FH], dt.bfloat16)
    wfc_d = din("wfc", [3 * FH, FO], dt.bfloat16)
    b1r_d = din("b1r", [1, FH], dt.bfloat16)
    b2r_d = din("b2r", [1, FH], dt.bfloat16)
    bfcr_d = din("bfcr", [1, FO], dt.bfloat16)
    sqdeg_d = din("sqdeg", [1, RPC], dt.bfloat16)
    dinv_d = din("dinv", [P, BPC], dt.float32)
    gidx_d = din("gidx", [P, IDXCOLS], dt.int16)
    counts_d = din("counts", [1, max(1, len(calls))], dt.int32)
    bpan_d = din("bpan", [P, NMM * P], dt.bfloat16)
    pm_d = din("pm", [P, BPC * G], dt.bfloat16)
    pmask_d = din("pmask", [P, G * BPC], dt.bfloat16)
    recip_d = din("recip", [G, 1], dt.float32)
    out_d = nc.dram_tensor("out", [G, FO], dt.float32,
                           kind="ExternalOutput").ap()

    rg = [list(range(n_cores))]

    from contextlib import ExitStack
    with tile.TileContext(nc) as tc, ExitStack() as ctx:
        const = ctx.enter_context(tc.tile_pool(name="const", bufs=1))
        dram = ctx.enter_context(tc.tile_pool(name="dram", bufs=1, space="DRAM"))
        tfpsum = ctx.enter_context(tc.tile_pool(name="tfpsum", bufs=2, space="PSUM"))
        aggpsum = ctx.enter_context(tc.tile_pool(name="aggpsum", bufs=3, space="PSUM"))
        tpsum = ctx.enter_context(tc.tile_pool(name="tpsum", bufs=1, space="PSUM"))
        spsum = ctx.enter_context(tc.tile_pool(name="spsum", bufs=1, space="PSUM"))
        fcpsum = ctx.enter_context(tc.tile_pool(name="fcpsum", bufs=1, space="PSUM"))
        msgp = ctx.enter_context(tc.tile_pool(name="msgp", bufs=5))
        bpp = ctx.enter_context(tc.tile_pool(name="bpp", bufs=2))
        btp = ctx.enter_context(tc.tile_pool(name="btp", bufs=4))
        hp = ctx.enter_context(tc.tile_pool(name="hp", bufs=3))
        htp = ctx.enter_context(tc.tile_pool(name="htp", bufs=4))
        tailp = ctx.enter_context(tc.tile_pool(name="tailp", bufs=1))

        # ---------------- constants into SBUF ----------------
        def cload(tag, dram_ap, shape, dtype):
            t = const.tile(shape, dtype, tag=tag)
            nc.sync.dma_start(out=t[:], in_=dram_ap)
            return t

        # trigger the first (dummy) collective ASAP: the runtime's one-time
        # collective init (~60us) runs serially before the first real
        # AllGather otherwise
        warm_in = dram.tile([8, 16], dt.bfloat16, tag="warmin")
        warm_out = dram.tile([64, 16], dt.bfloat16, tag="warmout",
                             addr_space="Shared")
        wz = const.tile([8, 16], dt.bfloat16, tag="wz")
        nc.gpsimd.memset(wz[:], 0.0)
        nc.sync.dma_start(out=warm_in[:], in_=wz[:])
        nc.gpsimd.collective_compute(
            "AllGather", mybir.AluOpType.bypass,
            ins=[warm_in[:].opt()], outs=[warm_out[:].opt()],
            replica_groups=rg)

        # zero-fill the msg ring once (skipped -1 pad slots must read
        # finite); allocated here so the ring sits at low SBUF offsets
        for _ in range(5):
            mb0 = msgp.tile([P, MAXT * FH], dt.bfloat16, tag="msg")
            nc.gpsimd.memset(mb0[:], 0.0)
        # pin the panel ring low as well (touch-allocations, ramp-hidden)
        for _ in range(2):
            bs0 = bpp.tile([P, MAXM * P], dt.bfloat16, tag="bsl")
            nc.gpsimd.memset(bs0[:], 0.0)

        gidx_sb = const.tile([P, IDXCOLS], dt.int16, tag="gidx")
        counts_sb = const.tile([1, max(1, len(calls))], dt.int32,
                               tag="counts")
        xt_sb = const.tile([P, KC * RPC], dt.bfloat16, tag="xt")
        for c in range(KC):
            nc.sync.dma_start(out=xt_sb[:, c * RPC:(c + 1) * RPC],
                              in_=xt_d[c * P:(c + 1) * P, :])
        w_sb = []
        for tag, d in (("w1", w1_d), ("w2", w2_d)):
            t = const.tile([P, KC * FH], dt.bfloat16, tag=tag)
            for c in range(KC):
                nc.sync.dma_start(out=t[:, c * FH:(c + 1) * FH],
                                  in_=d[c * P:(c + 1) * P, :])
            w_sb.append(t)
        wfc_sb = const.tile([P, FCK * FO], dt.bfloat16, tag="wfc")
        for c in range(FCK):
            nc.sync.dma_start(out=wfc_sb[:, c * FO:(c + 1) * FO],
                              in_=wfc_d[c * P:(c + 1) * P, :])
        dinv_sb = cload("dinv", dinv_d, [P, BPC], dt.float32)

        iota_sb = const.tile([P, P], dt.float32, tag="iota")
        nc.gpsimd.iota(out=iota_sb[:], pattern=[[1, P]], base=0,
                       channel_multiplier=0,
                       allow_small_or_imprecise_dtypes=True)
        iotac_sb = const.tile([P, 1], dt.float32, tag="iotac")
        nc.gpsimd.iota(out=iotac_sb[:], pattern=[[0, 1]], base=0,
                       channel_multiplier=1,
                       allow_small_or_imprecise_dtypes=True)
        ident_sb = const.tile([P, P], dt.bfloat16, tag="ident")
        nc.vector.tensor_tensor(out=ident_sb[:],
                                in0=iotac_sb[:].to_broadcast([P, P]),
                                in1=iota_sb[:],
                                op=mybir.AluOpType.is_equal)
        ones_sb = const.tile([1, G], dt.bfloat16, tag="ones")
        nc.gpsimd.memset(ones_sb[:], 1.0)
        blockmax = const.tile([P, KC * BPC], dt.bfloat16, tag="bmax")
        nc.gpsimd.memset(blockmax[:], 0.0)
        # persistent per-layer local tables (bf16, dinv-scaled)
        tbl = [const.tile([P, BPC * FH], dt.bfloat16, name=f"tbl{l}",
                          tag=f"tbl{l}")
               for l in range(2)]

        # DRAM bounce buffers for collectives (per layer, per half)
        ag_in = [[dram.tile([HROWS, FH], dt.bfloat16, name=f"agin{l}{h}",
                            tag=f"agin{l}{h}")
                  for h in (0, 1)] for l in range(2)]
        ag_out = [[dram.tile([HROWS * n_cores, FH], dt.bfloat16,
                             name=f"agout{l}{h}", tag=f"agout{l}{h}")
                   for h in (0, 1)] for l in range(2)]
        ars_in = [dram.tile([G, FH], dt.bfloat16, name=f"arsin{k}",
                            tag=f"arsin{k}") for k in (0, 1)]
        ars_out = [dram.tile([G, FH], dt.bfloat16, name=f"arsout{k}",
                             tag=f"arsout{k}", addr_space="Shared")
                   for k in (0, 1)]
        arm_in = dram.tile([P, KC * G], dt.bfloat16, tag="armin")
        arm_out = dram.tile([P, KC * G], dt.bfloat16, tag="armout",
                            addr_space="Shared")

        Copy = mybir.ActivationFunctionType.Copy
        Relu = mybir.ActivationFunctionType.Relu

        cnt_reg = nc.gpsimd.alloc_register("cntreg")

        def allgather(l, h):
            nc.gpsimd.collective_compute(
                "AllGather", mybir.AluOpType.bypass,
                ins=[ag_in[l][h][:].opt()], outs=[ag_out[l][h][:].opt()],
                replica_groups=rg)

        def push_block(l, b):
            h = 0 if b < HB else 1
            r0 = (b - h * HB) * P
            nc.scalar.dma_start(out=ag_in[l][h][r0:r0 + P, :],
                                in_=tbl[l][:, b * FH:(b + 1) * FH])

        # ---------------- layer-1 transform ----------------
        for b in range(BPC):
            ps = tfpsum.tile([P, FH], dt.float32, tag="tfps")
            for c in range(KC):
                nc.tensor.matmul(
                    out=ps[:],
                    lhsT=xt_sb[:, c * RPC + b * P:c * RPC + (b + 1) * P],
                    rhs=w_sb[0][:, c * FH:(c + 1) * FH],
                    start=(c == 0), stop=(c == KC - 1))
            nc.scalar.activation(out=tbl[0][:, b * FH:(b + 1) * FH],
                                 in_=ps[:], func=Copy,
                                 scale=dinv_sb[:, b:b + 1])
            push_block(0, b)
            if b == HB - 1:
                allgather(0, 0)
        allgather(0, 1)

        # deferred const loads (needed from the agg phase on, not by the
        # transforms -- keeps the early sync-DMA queue clear so the first
        # AllGather can fire as soon as the transforms finish)
        nc.sync.dma_start(out=gidx_sb[:], in_=gidx_d)
        nc.sync.dma_start(out=counts_sb[:], in_=counts_d)
        b1r_sb = cload("b1r", b1r_d, [1, FH], dt.bfloat16)
        b2r_sb = cload("b2r", b2r_d, [1, FH], dt.bfloat16)
        bfcr_sb = cload("bfcr", bfcr_d, [1, FO], dt.bfloat16)
        sqdeg_sb = cload("sqdeg", sqdeg_d, [1, RPC], dt.bfloat16)
        pm_sb = cload("pm", pm_d, [P, BPC * G], dt.bfloat16)
        pmask_sb = cload("pmask", pmask_d, [P, G * BPC], dt.bfloat16)
        recip_sb = cload("recip", recip_d, [G, 1], dt.float32)

        # ---------------- aggregation over edges ----------------
        AWIN = 3  # h=0 calls issued ahead of h=1 calls (hides AG of half 1)

        def agg_layer(l, bias_row, has_bias, produce_block):
            # interleave calls: [g0A g1A g2A g0B g1B ...] with groups'
            # matmul work following once both halves of a group are in.
            mbufs = {}   # call index -> sbuf tile

            def do_call(ci):
                call = calls[ci]
                T = call["T"]
                nc.gpsimd.reg_load(cnt_reg, counts_sb[0:1, ci:ci + 1])
                mb = msgp.tile([P, MAXT * FH], dt.bfloat16, tag="msg")
                out_ap = mb[:, :T * FH].rearrange("p (t e) -> p t e", e=FH)
                nc.gpsimd.dma_gather(
                    out_ap=out_ap,
                    in_ap=ag_out[l][call["h"]][:],
                    idxs_ap=gidx_sb[:, call["idx_off"]:
                                    call["idx_off"] + T * 8],
                    num_idxs=T * P,
                    num_idxs_reg=cnt_reg,
                    elem_size=FH,
                    single_packet=False)
                mbufs[ci] = mb

            order = []   # call issue order (A-window interleave)
            apend = [ci for ci in range(len(calls)) if calls[ci]["h"] == 0]
            bpend = [ci for ci in range(len(calls)) if calls[ci]["h"] == 1]
            ai = bi = 0
            while ai < len(apend) or bi < len(bpend):
                win = AWIN + 1 if bi == 0 else AWIN
                if ai < len(apend) and (ai - bi < win or bi >= len(bpend)):
                    order.append(apend[ai]); ai += 1
                else:
                    order.append(bpend[bi]); bi += 1

            done_upto = 0  # groups fully processed

            def group_ready(g):
                for h in (0, 1):
                    ci = call_of.get((g, h))
                    if ci is not None and ci not in mbufs:
                        return False
                return True

            def process_group(g):
                nmm = len(grp_mms[g])
                moff = grp_moff[g]
                bsl = None
                if nmm:
                    bsl = bpp.tile([P, MAXM * P], dt.bfloat16, tag="bsl")
                    nc.sync.dma_start(
                        out=bsl[:, :nmm * P],
                        in_=bpan_d[:, moff * P:(moff + nmm) * P])
                for pig in range(GROUP_NBLK):
                    b = g * GROUP_NBLK + pig
                    ps = aggpsum.tile([P, FH], dt.float32, tag="aggps")
                    mms = [e for e in grp_mms[g] if e[3] == pig]
                    # self loop: identity matmul against local table block
                    nc.tensor.matmul(out=ps[:], lhsT=ident_sb[:],
                                     rhs=tbl[l][:, b * FH:(b + 1) * FH],
                                     start=True,
                                     stop=(not mms) and not has_bias)
                    for k, (ci, t, gt, _pig, m) in enumerate(mms):
                        mo = m - moff
                        nc.tensor.matmul(
                            out=ps[:], lhsT=bsl[:, mo * P:(mo + 1) * P],
                            rhs=mbufs[ci][:, t * FH:(t + 1) * FH],
                            start=False,
                            stop=(k == len(mms) - 1) and not has_bias)
                    if has_bias:
                        nc.tensor.matmul(
                            out=ps[:],
                            lhsT=sqdeg_sb[:, b * P:(b + 1) * P],
                            rhs=bias_row[:],
                            start=False, stop=True)
                    produce_block(b, ps)

            for ci in order:
                do_call(ci)
                # process any groups that are now complete, in order
                while done_upto < NGRP and group_ready(done_upto):
                    process_group(done_upto)
                    g = done_upto
                    done_upto += 1
                    # release msg buffers of this group
                    for h in (0, 1):
                        cix = call_of.get((g, h))
                        if cix in mbufs:
                            del mbufs[cix]
                    if l == 0 and g == min(NGRP - 1, HB // GROUP_NBLK + 2):
                        allgather(1, 0)
            while done_upto < NGRP:
                process_group(done_upto)
                done_upto += 1
            if l == 0:
                allgather(1, 1)

        # layer-1 block epilogue: relu, transform to layer-2 table
        def produce1(b, ps):
            h1 = hp.tile([P, FH], dt.bfloat16, tag="h1")
            nc.scalar.activation(out=h1[:], in_=ps[:], func=Relu,
                                 scale=dinv_sb[:, b:b + 1])
            h1t = []
            for c in range(KC):
                tp = tpsum.tile([P, P], dt.bfloat16, tag="tp")
                nc.tensor.transpose(out=tp[:],
                                    in_=h1[:, c * P:(c + 1) * P],
                                    identity=ident_sb[:])
                ht = htp.tile([P, P], dt.bfloat16, tag="ht")
                nc.vector.tensor_copy(out=ht[:], in_=tp[:])
                h1t.append(ht)
            ps2 = tfpsum.tile([P, FH], dt.float32, tag="tfps")
            for c in range(KC):
                nc.tensor.matmul(out=ps2[:], lhsT=h1t[c][:],
                                 rhs=w_sb[1][:, c * FH:(c + 1) * FH],
                                 start=(c == 0), stop=(c == KC - 1))
            nc.scalar.activation(out=tbl[1][:, b * FH:(b + 1) * FH],
                                 in_=ps2[:], func=Copy,
                                 scale=dinv_sb[:, b:b + 1])
            push_block(1, b)

        agg_layer(0, b1r_sb, plan["has_b1"], produce1)

        # layer-2 block epilogue: relu, pooling contributions
        sums_ps = spsum.tile([G, FH], dt.float32, tag="sums")
        mxT_loc = const.tile([P, KC * G], dt.bfloat16, tag="mxT_loc")

        SUMS_SPLIT = BPC - 6 if BPC > 8 else -1  # no split on tiny configs

        def produce2(b, ps):
            h2 = hp.tile([P, FH], dt.bfloat16, tag="h2")
            nc.scalar.activation(out=h2[:], in_=ps[:], func=Relu,
                                 scale=dinv_sb[:, b:b + 1])
            nc.tensor.matmul(out=sums_ps[:],
                             lhsT=pm_sb[:, b * G:(b + 1) * G],
                             rhs=h2[:],
                             start=(b == 0 or b == SUMS_SPLIT),
                             stop=(b == SUMS_SPLIT - 1 or b == BPC - 1))
            if b == SUMS_SPLIT - 1:
                ssA = tailp.tile([G, FH], dt.bfloat16, tag="ssA")
                nc.vector.tensor_copy(out=ssA[:], in_=sums_ps[:])
                nc.sync.dma_start(out=ars_in[0][:], in_=ssA[:])
                nc.gpsimd.collective_compute(
                    "AllReduce", mybir.AluOpType.add,
                    ins=[ars_in[0][:].opt()], outs=[ars_out[0][:].opt()],
                    replica_groups=rg)
            for c in range(KC):
                tp = tpsum.tile([P, P], dt.bfloat16, tag="tp")
                nc.tensor.transpose(out=tp[:],
                                    in_=h2[:, c * P:(c + 1) * P],
                                    identity=ident_sb[:])
                nc.vector.tensor_reduce(
                    out=blockmax[:, c * BPC + b:c * BPC + b + 1],
                    in_=tp[:], axis=mybir.AxisListType.X,
                    op=mybir.AluOpType.max)
            # per-graph local max for graphs whose last block is b
            for g in graphs_at[b]:
                mtmp = btp.tile([P, BPC], dt.bfloat16, tag="mtmp")
                for c in range(KC):
                    nc.vector.tensor_tensor(
                        out=mtmp[:],
                        in0=blockmax[:, c * BPC:(c + 1) * BPC],
                        in1=pmask_sb[:, g * BPC:(g + 1) * BPC],
                        op=mybir.AluOpType.mult)
                    nc.vector.tensor_reduce(
                        out=mxT_loc[:, c * G + g:c * G + g + 1],
                        in_=mtmp[:],
                        axis=mybir.AxisListType.X, op=mybir.AluOpType.max)

        agg_layer(1, b2r_sb, plan["has_b2"], produce2)

        # ---------------- pooling tail ----------------
        sums_sb = tailp.tile([G, FH], dt.bfloat16, tag="sums_sb")
        nc.vector.tensor_copy(out=sums_sb[:], in_=sums_ps[:])
        nc.sync.dma_start(out=ars_in[1][:], in_=sums_sb[:])
        nc.gpsimd.collective_compute(
            "AllReduce", mybir.AluOpType.add,
            ins=[ars_in[1][:].opt()], outs=[ars_out[1][:].opt()],
            replica_groups=rg)
        nc.sync.dma_start(out=arm_in[:], in_=mxT_loc[:])
        nc.gpsimd.collective_compute(
            "AllReduce", mybir.AluOpType.max,
            ins=[arm_in[:].opt()], outs=[arm_out[:].opt()],
            replica_groups=rg)

        gsA = tailp.tile([G, FH], dt.bfloat16, tag="gsA")
        if SUMS_SPLIT > 0:
            nc.sync.dma_start(out=gsA[:], in_=ars_out[0][:])
        else:
            nc.gpsimd.memset(gsA[:], 0.0)
        gsB = tailp.tile([G, FH], dt.bfloat16, tag="gsB")
        nc.sync.dma_start(out=gsB[:], in_=ars_out[1][:])
        gsums = tailp.tile([G, FH], dt.bfloat16, tag="gsums")
        nc.vector.tensor_tensor(out=gsums[:], in0=gsA[:], in1=gsB[:],
                                op=mybir.AluOpType.add)
        mxT = tailp.tile([P, KC * G], dt.bfloat16, tag="mxT")
        nc.sync.dma_start(out=mxT[:], in_=arm_out[:])

        # mean / sums in bf16, transposed to feature-major for the FC
        mean_sb = tailp.tile([G, FH], dt.bfloat16, tag="mean")
        nc.vector.tensor_scalar(out=mean_sb[:], in0=gsums[:],
                                scalar1=recip_sb[:], scalar2=None,
                                op0=mybir.AluOpType.mult)
        sums_bf = tailp.tile([G, FH], dt.bfloat16, tag="sumsbf")
        nc.vector.tensor_copy(out=sums_bf[:], in_=gsums[:])
        meanT = tailp.tile([P, KC * G], dt.bfloat16, tag="meanT")
        sumsT = tailp.tile([P, KC * G], dt.bfloat16, tag="sumsT")
        for src, dst_t in ((mean_sb, meanT), (sums_bf, sumsT)):
            for c in range(KC):
                tp = tpsum.tile([P, P], dt.bfloat16, tag="tp")
                nc.tensor.transpose(out=tp[:, :G],
                                    in_=src[:, c * P:(c + 1) * P],
                                    identity=ident_sb[:G, :G])
                nc.vector.tensor_copy(out=dst_t[:, c * G:(c + 1) * G],
                                      in_=tp[:, :G])

        # final FC: out = [mean | max | sums] @ Wfc + bfc
        fc_ps = fcpsum.tile([G, FO], dt.float32, tag="fc")
        gT = [meanT, mxT, sumsT]
        k = 0
        for part in range(3):
            for c in range(KC):
                nc.tensor.matmul(
                    out=fc_ps[:], lhsT=gT[part][:, c * G:(c + 1) * G],
                    rhs=wfc_sb[:, k * FO:(k + 1) * FO],
                    start=(k == 0),
                    stop=(k == FCK - 1) and not plan["has_bfc"])
                k += 1
        if plan["has_bfc"]:
            nc.tensor.matmul(out=fc_ps[:], lhsT=ones_sb[:], rhs=bfcr_sb[:],
                             start=False, stop=True)
        out_sb = tailp.tile([G, FO], dt.float32, tag="out_sb")
        nc.vector.tensor_copy(out=out_sb[:], in_=fc_ps[:])
        nc.sync.dma_start(out=out_d[:], in_=out_sb[:])

    nc.compile()
    return nc


# --------------------------------------------------------------------------
# Entry point for the grading harness.
# --------------------------------------------------------------------------

def kernel(x, edge_index, batch, n_graphs, W1, b1, W2, b2, Wfc, bfc,
           **_unused):
    plan, in_maps = preprocess(x, edge_index, batch, n_graphs,
                               W1, b1, W2, b2, Wfc, bfc)
    nc = build(plan)
    res = run_bass_kernel_spmd(nc, in_maps, core_ids=list(range(NCORES)))
    out = np.asarray(res.results[0]["out"], np.float32)
    return out
